# revision 26
# baseline (speedup 1.0000x reference)
"""Trainium2 Bass kernel: LookupTransformerBlock (block-causal sparse attention).

Reference semantics (B=4, T=784, D=768, H=12, Dh=64, d_ff=3072):
  x_aug = LN1(concat(memory[:, :T], x))              # [B, 2T, D], ln1 g=1/b=0
  h     = LN_att(x_aug)  (== x_aug up to O(eps) since x_aug is normalized)
  qkv   = h @ w_qkv.T ; block-causal attention over frames of 196
  x2    = x_aug + attn_out
  out   = (x2 + FFN(LN2(x2)))[:, T:, :]

Sharding: 8 cores = (batch b in 0..3) x (frame-half hf in 0..1); each core's
392 query rows are rows [hf*98, hf*98+98) of each of the 4 x-frames, ordered
[f4|f5|f6|f7].  K/V over all 1568 positions (data parallel, no collectives).

v3 design (vs bf16 v2 at 291us):
  - fp8e4m3 DoubleRow matmuls (2x PE throughput, HW-verified) for the K, V,
    Q GEMMs, the PV accumulation and the out-projection.  QK scores and the
    FFN stay bf16 (FFN fp8 measured at 3e-2 rel err vs the 2e-2 budget).
  - weight blocks are scaled x4 host-side (w_qkv entries ~N(0,1/768) sit in
    the fp8 subnormal range); the factors fold into existing scale operands:
    Q*K x16 into the exp scale, V x4 into the rs/4 evacuation scalar,
    out-proj (x4 w, x16 ONT) into a 1/64 evac multiply.
  - exp shift -4.0 keeps exp scores inside fp8 range (max |dots| = 8.8).
  - frame-ordered queries make the mask structure uniform across cores:
    score/PV matmuls shrink widths on high j-tiles (19% less QK/PV/exp), and
    the only masking left is three static 98-col zero passes per head pair.
  - the 12 per-head softmax denominators are DMA-gathered into a [12, NQ]
    tile for one 12-lane reciprocal (was 12 single-lane 2us reciprocals).
"""

import os
import sys
from contextlib import ExitStack

import numpy as np
import ml_dtypes

for _p in ("/opt/trn_rl_repo", os.path.expanduser("~/.axon_site/_ro/trn_rl_repo")):
    if os.path.isdir(_p) and _p not in sys.path:
        sys.path.append(_p)

import concourse.bass as bass
import concourse.bacc as bacc
import concourse.mybir as mybir
import concourse.tile as tile
from concourse.bass_utils import run_bass_kernel_spmd
from concourse.masks import make_identity

F32 = mybir.dt.float32
BF16 = mybir.dt.bfloat16
FP8 = mybir.dt.float8e4
DR = mybir.MatmulPerfMode.DoubleRow
AF = mybir.ActivationFunctionType
ALU = mybir.AluOpType

B = 4
T = 784
D = 768
L = 2 * T            # 1568
NQ = 392             # query rows per core
H = 12
DH = 64
DFF = 3072
NPATCH = 196
DC = D // 128        # 6
FT = DFF // 128      # 24
NJT = 13             # j-tiles over L (12 x 128 + 32)
JSZ = [128] * 12 + [32]
LCH = [512, 512, 512, 32]
EPS = 1e-5
NCORES = 8

# query-width tables (frame-ordered queries: cols [f4|f5|f6|f7] x 98)
OFFW = [(0, 392)] * 8 + [(98, 294), (98, 294), (196, 196), (294, 98), (294, 98)]
# jt -> (first masked key within tile, query col of the partial frame,
#        consts column holding the 0/1 key mask)
ZEROPASS = {7: (84, 0, 0), 9: (24, 98, 1), 10: (92, 196, 2)}
PAIRW = [(0, 392), (0, 392), (0, 392), (0, 392), (98, 294), (196, 196)]

# consts column layout
C_CBQ = 0            # 6:  4 * (w_qkv[:D] @ b_att)
C_B2 = 6             # 6:  b2
C_CB1 = 12           # 24: w1_eff bias (x1; FFN is bf16)
C_SCO = 36           # 1:  1/64  (out-proj evac scale)
C_BB = 37            # 1:  -4.0  (exp shift)
C_ZP = 38            # 3:  0/1 key masks for the j-tiles with a frame boundary
NCONST = 41

QK_SCALE = DH ** -0.5
# smt = (4Q)(4K) = 16*dots_unnorm; exp scale must be rs*qscale/16 and the
# rs columns hold rs/4, so the constant factor is qscale/4.
SC_MUL = QK_SCALE / 4.0


def _ln_stats_bf(nc, pst, psq, env, xblocks, lch):
    """bf16 column stats (used for the q-slice): sum and sum-of-squares."""
    ones_col = env["ones_col"]
    mu_ps = pst.tile([1, lch], F32, tag="mu", name="mu_ps")
    msq_ps = pst.tile([1, lch], F32, tag="ms", name="msq_ps")
    for dc in range(DC):
        nc.tensor.matmul(mu_ps[:], lhsT=ones_col[:], rhs=xblocks[dc],
                         start=(dc == 0), stop=(dc == DC - 1))
    for dc in range(DC):
        sq = psq.tile([128, lch], BF16, tag="sq", name="sq")
        if dc % 2 == 0:
            nc.scalar.square(sq[:], xblocks[dc])
        else:
            nc.gpsimd.tensor_mul(sq[:], xblocks[dc], xblocks[dc])
        nc.tensor.matmul(msq_ps[:], lhsT=ones_col[:], rhs=sq[:],
                         start=(dc == 0), stop=(dc == DC - 1))
    return mu_ps, msq_ps


def _ln_stats_fp8(nc, pst, psq, env, x8c, lch):
    """fp8 chunk stats: mu via DoubleRow over dc pairs, msq via bf16 squares."""
    ones_col = env["ones_col"]
    ones8 = env["ones8"]
    mu_ps = pst.tile([1, lch], F32, tag="mu", name="mu_ps")
    msq_ps = pst.tile([1, lch], F32, tag="ms", name="msq_ps")
    for dc in range(DC):
        nc.tensor.matmul(mu_ps[:], lhsT=ones8[:, 0:1], rhs=x8c[:, dc, :],
                         start=(dc == 0), stop=(dc == DC - 1))
    for dc in range(DC):
        sq = psq.tile([128, lch], BF16, tag="sq", name="sq")
        if dc % 2 == 0:
            nc.scalar.square(sq[:], x8c[:, dc, :])
        else:
            nc.gpsimd.tensor_mul(sq[:], x8c[:, dc, :], x8c[:, dc, :])
        nc.tensor.matmul(msq_ps[:], lhsT=ones_col[:], rhs=sq[:],
                         start=(dc == 0), stop=(dc == DC - 1))
    return mu_ps, msq_ps


def _ln_rows(nc, prow, env, mu_ps, msq_ps, lch, do_rcp=True, sd_scale=1.0):
    """mu/sd row math; sd_scale>1 bakes a constant into sd (so downstream
    reciprocals produce rs/sd_scale)."""
    mu_bf = prow.tile([1, lch], BF16, tag="mubf", name="mu_bf")
    nc.scalar.mul(mu_bf[:], mu_ps[:], 1.0 / D)
    msq = prow.tile([1, lch], F32, tag="msq", name="msq")
    nc.scalar.mul(msq[:], msq_ps[:], 1.0 / D)
    var = prow.tile([1, lch], F32, tag="var", name="var")
    nc.gpsimd.tensor_mul(var[:], mu_bf[:], mu_bf[:])
    nc.gpsimd.tensor_sub(var[:], msq[:], var[:])
    sd = prow.tile([1, lch], F32, tag="sd", name="sd")
    s2 = sd_scale * sd_scale
    eps_ap = env["eps16"] if sd_scale == 4.0 else env["eps1"]
    nc.scalar.activation(sd[:], var[:], AF.Sqrt, bias=eps_ap[0:1, 0:1], scale=s2)
    if not do_rcp:
        return mu_bf, sd, None
    rs_t = prow.tile([1, lch], F32, tag="rsf", name="rs_f")
    nc.vector.reciprocal(rs_t[:], sd[:])
    rs_bf = prow.tile([1, lch], BF16, tag="rsbf", name="rs_bf")
    nc.scalar.mul(rs_bf[:], rs_t[:], 1.0)
    return mu_bf, rs_t, rs_bf


def build_program():
    nc = bacc.Bacc("TRN2")
    xall_d = nc.declare_dram_parameter("xall", [128, DC * L], FP8, isOutput=False)
    xq_d = nc.declare_dram_parameter("xq", [128, DC * NQ], BF16, isOutput=False)
    wq_d = nc.declare_dram_parameter("wq", [128, DC * 3 * D], FP8, isOutput=False)
    wout_d = nc.declare_dram_parameter("wout", [128, DC * D], FP8, isOutput=False)
    w1_d = nc.declare_dram_parameter("w1", [128, DC * DFF], BF16, isOutput=False)
    w2_d = nc.declare_dram_parameter("w2", [128, FT * D], BF16, isOutput=False)
    consts_d = nc.declare_dram_parameter("consts", [128, NCONST], F32, isOutput=False)
    bvec_d = nc.declare_dram_parameter("bvec", [1, D], BF16, isOutput=False)
    out_d = nc.declare_dram_parameter("out", [NQ, D], F32, isOutput=True)

    env = {}
    with tile.TileContext(nc) as tc, ExitStack() as top:
        pc = top.enter_context(tc.tile_pool(name="const", bufs=1))
        consts = pc.tile([128, NCONST], F32, tag="consts", name="consts")
        nc.gpsimd.dma_start(consts[:], consts_d[:])
        bvec = pc.tile([1, D], BF16, tag="bvec", name="bvec")
        nc.gpsimd.dma_start(bvec[:], bvec_d[:])
        ones_col = pc.tile([128, 1], BF16, tag="onc", name="ones_col")
        nc.vector.memset(ones_col[:], 1.0)
        ones_colf = pc.tile([128, 1], F32, tag="oncf", name="ones_colf")
        nc.vector.memset(ones_colf[:], 1.0)
        ones_row = pc.tile([1, 128], BF16, tag="onr", name="ones_row")
        nc.vector.memset(ones_row[:], 1.0)
        ones_rowf = pc.tile([1, 128], F32, tag="onrf", name="ones_rowf")
        nc.vector.memset(ones_rowf[:], 1.0)
        ones_rq = pc.tile([1, NQ], BF16, tag="onrq", name="ones_rq")
        nc.vector.memset(ones_rq[:], 1.0)
        ones128 = pc.tile([128, 64], BF16, tag="on128", name="ones128")
        nc.vector.memset(ones128[:], 1.0)
        ones8 = pc.tile([128, 2], FP8, tag="on8", name="ones8")
        nc.vector.memset(ones8[:], 1.0)
        eps1 = pc.tile([1, 1], F32, tag="eps", name="eps1")
        nc.vector.memset(eps1[:], EPS)
        eps16 = pc.tile([1, 1], F32, tag="eps16", name="eps16")
        nc.vector.memset(eps16[:], EPS * 16.0)
        ident = pc.tile([128, 128], F32, tag="ident", name="ident")
        make_identity(nc, ident[:])
        # per-LN-chunk rs/4 columns (token-major) and exp scales
        rsc_c = [pc.tile([128, 4], F32, tag=f"rsc{ci}", name=f"rsc{ci}")
                 for ci in range(4)]
        sc_c = [pc.tile([128, 4], F32, tag=f"sc{ci}", name=f"sc{ci}")
                for ci in range(4)]
        env.update(ones_col=ones_col, ones8=ones8, eps1=eps1, eps16=eps16)

        def rs_col(jt, psz):
            return rsc_c[jt // 4][0:psz, jt % 4:jt % 4 + 1]

        def sc_col(jt, psz):
            return sc_c[jt // 4][0:psz, jt % 4:jt % 4 + 1]

        # chunk-major xall layout: chunk ci holds DC stripes of width LCH[ci]
        XC0 = [0, 3072, 6144, 9216]

        pnq = top.enter_context(tc.tile_pool(name="nqp", bufs=1))
        nqT = pnq.tile([128, DC * NQ], BF16, tag="nq", name="nqT")
        nq8 = pnq.tile([128, DC, NQ], FP8, tag="nq8", name="nq8")
        px2 = top.enter_context(tc.tile_pool(name="x2p", bufs=DC))
        x2T = [px2.tile([128, NQ], F32, tag="x2", name=f"x2T{i}") for i in range(DC)]
        pont = top.enter_context(tc.tile_pool(name="ontp", bufs=1))
        ONT8 = pont.tile([128, DC, NQ], FP8, tag="ont", name="ONT8")
        pn2 = top.enter_context(tc.tile_pool(name="n2p", bufs=DC))
        n2T = [pn2.tile([128, NQ], BF16, tag="n2", name=f"n2T{i}") for i in range(DC)]
        pout = top.enter_context(tc.tile_pool(name="outp", bufs=DC))
        outT = [pout.tile([128, NQ], F32, tag="ot", name=f"outT{i}") for i in range(DC)]
        prow = top.enter_context(tc.tile_pool(name="rows", bufs=1))
        pwA = top.enter_context(tc.tile_pool(name="wAp", bufs=1))
        wout = pwA.tile([128, DC, D], FP8, tag="wo", name="wout")
        posb = top.enter_context(tc.tile_pool(name="osbp", bufs=2 * DC))
        s_att = ExitStack()   # attention-lifetime tiles; freed before w1/w2
        pkt = s_att.enter_context(tc.tile_pool(name="ktp", bufs=DC))
        KT = [pkt.tile([128, L], BF16, tag="kt", name=f"KT{i}") for i in range(DC)]
        pqt = s_att.enter_context(tc.tile_pool(name="qtp", bufs=DC))
        QT = [pqt.tile([128, NQ], BF16, tag="qt", name=f"QT{i}") for i in range(DC)]
        pva = s_att.enter_context(tc.tile_pool(name="vap", bufs=6))
        VA8 = [pva.tile([128, 2, H, 72], FP8, tag="va", name=f"VA8_{i}")
               for i in range(6)]
        pvat = s_att.enter_context(tc.tile_pool(name="vatp", bufs=1))
        VA8t = pvat.tile([32, 1, H, 72], FP8, tag="vat", name="VA8t")
        ppt = s_att.enter_context(tc.tile_pool(name="ptp", bufs=3))
        pptt = s_att.enter_context(tc.tile_pool(name="pttp", bufs=2))

        # short-lived inputs on the right allocator stack (freed mid-program)
        s_qkv = ExitStack()   # xall (rewritten in place to x-mu), wq
        s_ln = ExitStack()    # xq + square scratch; dies after attention setup

        pqkv = s_qkv.enter_context(tc.tile_pool(name="qkvp", bufs=1, side="right"))
        xall = pqkv.tile([128, DC * L], FP8, tag="xa", name="xall")
        wq = pqkv.tile([128, DC, 3 * D], FP8, tag="wq", name="wq")
        pxq = s_ln.enter_context(tc.tile_pool(name="xqp", bufs=1, side="right"))
        xq = pxq.tile([128, DC * NQ], BF16, tag="xq", name="xq")
        nc.sync.dma_start(xall[:, 0:1536], xall_d[:, 0:1536])
        nc.sync.dma_start(xall[:, 1536:3072], xall_d[:, 1536:3072])
        for ci in range(1, 3):
            nc.sync.dma_start(xall[:, XC0[ci]:XC0[ci + 1]],
                              xall_d[:, XC0[ci]:XC0[ci + 1]])
        nc.sync.dma_start(xq[:], xq_d[:])
        nc.sync.dma_start(xall[:, XC0[3]:DC * L], xall_d[:, XC0[3]:DC * L])
        nc.sync.dma_start(wq[:], wq_d[:].rearrange("p (k c) -> p k c", k=DC))

        # chunk views: x8c[ci] is [128, DC, lch]
        x8c = [xall[:, XC0[ci]:XC0[ci] + DC * LCH[ci]].rearrange(
            "p (k c) -> p k c", k=DC) for ci in range(4)]

        for lt in range(6):
            nc.gpsimd.memset(VA8[lt][:], 1.0 / 16.0)
        nc.gpsimd.memset(VA8t[:], 1.0 / 16.0)
        nc.gpsimd.dma_start(wout[:], wout_d[:].rearrange("p (k c) -> p k c", k=DC))

        # ---------------- LN1 + Q GEMM ----------------
        xqb = [xq[:, dc * NQ:(dc + 1) * NQ] for dc in range(DC)]

        def emit_chunk_tail(ci, pbc, pst):
            lch = LCH[ci]
            mu_bf, sd, _ = _ln_rows(nc, prow, env, *stq[ci], lch,
                                    do_rcp=False, sd_scale=4.0)
            mub = pbc.tile([128, lch], F32, tag="bc", name="mub")
            nc.tensor.matmul(mub[:], lhsT=ones_row[:], rhs=mu_bf[:],
                             start=True, stop=True)
            for dc in range(DC):
                nc.vector.tensor_sub(x8c[ci][:, dc, :], x8c[ci][:, dc, :], mub[:])
            njc = 4 if ci < 3 else 1
            sdT_ps = pst.tile([128, 4], F32, tag="mu", name="sdT_ps")
            if ci == 3:
                nc.vector.memset(sdT_ps[:], 1.0)
            for k in range(njc):
                cnt = min(128, lch - k * 128)
                nc.tensor.matmul(sdT_ps[0:cnt, k:k + 1],
                                 lhsT=sd[0:1, k * 128:k * 128 + cnt],
                                 rhs=ones_rowf[0:1, 0:1],
                                 start=True, stop=True, skip_group_check=True)
            nc.vector.reciprocal(rsc_c[ci][:], sdT_ps[:])
            nc.scalar.mul(sc_c[ci][:], rsc_c[ci][:], SC_MUL)

        with ExitStack() as s:
            pst = s.enter_context(tc.tile_pool(name="stps", bufs=2, space="PSUM"))
            pbc = s.enter_context(tc.tile_pool(name="bcps", bufs=2, space="PSUM"))
            psv = s.enter_context(tc.tile_pool(name="vps", bufs=1, space="PSUM"))
            psq = s.enter_context(tc.tile_pool(name="sqp", bufs=3, side="right"))

            def emit_kv(ci):
                # K(et=0) for this chunk, then V for its j-tiles (fp8 DR)
                lch = LCH[ci]
                ps_k = pbc.tile([128, lch], F32, tag="bc", name="ps_k")
                for pp in range(3):
                    nc.tensor.matmul(
                        ps_k[:], lhsT=wq[:, 2 * pp:2 * pp + 2, D:D + 128],
                        rhs=x8c[ci][:, 2 * pp:2 * pp + 2, :],
                        start=(pp == 0), stop=(pp == 2), perf_mode=DR)
                nc.scalar.copy(KT[0][:, ci * 512:ci * 512 + lch], ps_k[:])
                for jt in range(4 * ci, min(4 * ci + 4, NJT)):
                    jsz = JSZ[jt]
                    o = (jt % 4) * 128
                    ps_v = psv.tile([128, D], F32, tag="psv", name="ps_v")
                    for pp in range(3):
                        lhsT = x8c[ci][:, 2 * pp:2 * pp + 2, o:o + jsz]
                        nc.tensor.matmul(ps_v[0:jsz, 0:512], lhsT=lhsT,
                                         rhs=wq[:, 2 * pp:2 * pp + 2, 2 * D:2 * D + 512],
                                         start=(pp == 0), stop=(pp == 2),
                                         perf_mode=DR, skip_group_check=True)
                        nc.tensor.matmul(ps_v[0:jsz, 512:D], lhsT=lhsT,
                                         rhs=wq[:, 2 * pp:2 * pp + 2, 2 * D + 512:3 * D],
                                         start=(pp == 0), stop=(pp == 2),
                                         perf_mode=DR, skip_group_check=True)
                    psvv = ps_v[0:jsz, :].rearrange("p (h c) -> p h c", c=64)
                    if jt < 12:
                        dst = VA8[jt // 2][0:jsz, jt % 2, :, 0:64]
                    else:
                        dst = VA8t[0:jsz, 0, :, 0:64]
                    nc.vector.tensor_scalar_mul(dst, psvv, rs_col(jt, jsz))

            stq = [None] * 5
            stq[0] = _ln_stats_fp8(nc, pst, psq, env, x8c[0], LCH[0])
            stq[1] = _ln_stats_fp8(nc, pst, psq, env, x8c[1], LCH[1])
            emit_chunk_tail(0, pbc, pst)
            stq[4] = _ln_stats_bf(nc, pst, psq, env, xqb, NQ)
            emit_chunk_tail(1, pbc, pst)
            stq[2] = _ln_stats_fp8(nc, pst, psq, env, x8c[2], LCH[2])

            # q slice: full normalize (mu and rs)
            mu_bfq, _, rs_bfq = _ln_rows(nc, prow, env, *stq[4], NQ)
            mubq = pbc.tile([128, NQ], F32, tag="bc", name="mubq")
            nc.tensor.matmul(mubq[:], lhsT=ones_row[:], rhs=mu_bfq[:],
                             start=True, stop=True)
            sbq = pbc.tile([128, NQ], F32, tag="bc", name="sbq")
            nc.tensor.matmul(sbq[:], lhsT=ones_row[:], rhs=rs_bfq[:],
                             start=True, stop=True)
            for dc in range(DC):
                tmp = psq.tile([128, NQ], F32, tag="tmq", name="tmq")
                nc.vector.tensor_sub(tmp[:], xqb[dc], mubq[:])
                nc.vector.tensor_mul(nqT[:, dc * NQ:(dc + 1) * NQ], tmp[:], sbq[:])
                nc.scalar.copy(nq8[:, dc, :], nqT[:, dc * NQ:(dc + 1) * NQ])

            # Q GEMM (fp8 DR over dc pairs)
            for et in range(DC):
                ps_q = pbc.tile([128, NQ], F32, tag="bc", name="ps_q")
                for pp in range(3):
                    nc.tensor.matmul(
                        ps_q[:],
                        lhsT=wq[:, 2 * pp:2 * pp + 2, et * 128:(et + 1) * 128],
                        rhs=nq8[:, 2 * pp:2 * pp + 2, :],
                        start=(pp == 0), stop=(pp == 2), perf_mode=DR)
                nc.scalar.activation(QT[et][:], ps_q[:], AF.Identity,
                                     bias=consts[:, C_CBQ + et:C_CBQ + et + 1])

            stq[3] = _ln_stats_fp8(nc, pst, psq, env, x8c[3], LCH[3])
            emit_chunk_tail(2, pbc, pst)
            emit_chunk_tail(3, pbc, pst)
            for ci in range(4):
                emit_kv(ci)
        s_ln.close()

        # ---------------- attention ----------------
        o_sbs = []
        with ExitStack() as s:
            psc = s.enter_context(tc.tile_pool(name="scps", bufs=2, space="PSUM"))
            pso = s.enter_context(tc.tile_pool(name="sops", bufs=1, space="PSUM"))
            pkk = s.enter_context(tc.tile_pool(name="kkps", bufs=2, space="PSUM"))

            kwork = {}  # hp -> list of (et, ci, pp) DR K matmul work items

            def emit_k_steps(hp, n):
                wl = kwork.get(hp)
                for _ in range(n):
                    if not wl:
                        return
                    et, ci, pp = wl.pop(0)
                    lch = LCH[ci]
                    if pp == 0:
                        kwork["ps"] = pkk.tile([128, lch], F32, tag="kk",
                                               name="ps_kk")
                    ps_k = kwork["ps"]
                    nc.tensor.matmul(
                        ps_k[:],
                        lhsT=wq[:, 2 * pp:2 * pp + 2,
                                D + et * 128:D + (et + 1) * 128],
                        rhs=x8c[ci][:, 2 * pp:2 * pp + 2, :],
                        start=(pp == 0), stop=(pp == 2),
                        perf_mode=DR, skip_group_check=True)
                    if pp == 2:
                        nc.vector.tensor_copy(KT[et][:, ci * 512:ci * 512 + lch],
                                              ps_k[:])

            def emit_pv(item):
                kind, hp, o_ps = item[0], item[1], o_ps_by_hp[item[1]]
                if kind == 'pair':
                    p, pt = item[2], item[3]
                    off, w = PAIRW[p]
                    for hi in range(2):
                        h = 2 * hp + hi
                        nc.tensor.matmul(
                            o_ps[hi][0:65, off:off + w],
                            lhsT=VA8[p][:, :, h, 0:65],
                            rhs=pt[:, :, hi, off:off + w],
                            start=(p == 0), stop=False,
                            perf_mode=DR, skip_group_check=True)
                else:
                    ptt = item[2]
                    for hi in range(2):
                        h = 2 * hp + hi
                        nc.tensor.matmul(
                            o_ps[hi][0:65, 294:392],
                            lhsT=VA8t[0:32, 0, h, 0:65],
                            rhs=ptt[0:32, hi, 294:392],
                            start=False, stop=True, skip_group_check=True)
                    for hi in range(2):
                        o_sb = posb.tile([65, NQ], BF16, tag="osb", name="o_sb")
                        nc.vector.tensor_copy(o_sb[:], o_ps[hi][0:65, :])
                        rrow = posb.tile([1, NQ], BF16, tag="rrb", name="rrow")
                        with nc.allow_low_precision(
                                reason="bf16 softmax denominators on a 2e-2 "
                                       "tolerance output"):
                            nc.vector.reciprocal(rrow[:], o_sb[64:65, :])
                        o_sbs.append((o_sb, rrow))

            seq = [(hp, p) for hp in range(DC) for p in range(7)]
            for hp in range(DC - 1):
                kwork[hp] = [(hp + 1, ci, pp) for ci in range(4)
                             for pp in range(3)]
            o_ps_by_hp = {}
            lags = []
            for hp, p in seq:
                if p == 0:
                    o_ps_by_hp[hp] = [
                        pso.tile([128, NQ], F32, tag=f"o{hi}", name=f"o_ps{hi}")
                        for hi in range(2)]
                if p < 6:
                    pt = ppt.tile([128, 2, 2, NQ], FP8, tag="pt", name="pt")
                    if p == 5:
                        nc.gpsimd.memset(pt[:, 1, :, 196:294], 0.0)
                    for sub in range(2):
                        jt = 2 * p + sub
                        jsz = JSZ[jt]
                        off, w = OFFW[jt]
                        smt = psc.tile([128, 1024], F32, tag="smt", name="smt")
                        for hi in range(2):
                            part = 64 * hi
                            nc.tensor.matmul(
                                smt[0:jsz, 512 * hi:512 * hi + w],
                                lhsT=KT[hp][part:part + 64,
                                            jt * 128:jt * 128 + jsz],
                                rhs=QT[hp][part:part + 64, off:off + w],
                                start=True, stop=True, skip_group_check=True)
                        emit_k_steps(hp, 1)
                        smt_v = smt[0:jsz].rearrange("p (b c) -> p b c", c=512)
                        nc.scalar.activation(
                            pt[0:jsz, sub, :, off:off + w],
                            smt_v[:, :, 0:w], AF.Exp,
                            bias=consts[0:jsz, C_BB:C_BB + 1],
                            scale=sc_col(jt, jsz))
                        if jt in ZEROPASS:
                            _, qoff, zc = ZEROPASS[jt]
                            nc.vector.tensor_scalar_mul(
                                pt[0:jsz, sub, :, qoff:qoff + 98],
                                pt[0:jsz, sub, :, qoff:qoff + 98],
                                consts[0:jsz, C_ZP + zc:C_ZP + zc + 1])
                    item = ('pair', hp, p, pt)
                else:
                    ptt = pptt.tile([32, 2, NQ], FP8, tag="ptt", name="ptt")
                    jsz = JSZ[12]
                    off, w = OFFW[12]
                    smt = psc.tile([128, 1024], F32, tag="smt", name="smt")
                    for hi in range(2):
                        part = 64 * hi
                        nc.tensor.matmul(
                            smt[0:jsz, 512 * hi:512 * hi + w],
                            lhsT=KT[hp][part:part + 64, 1536:1536 + jsz],
                            rhs=QT[hp][part:part + 64, off:off + w],
                            start=True, stop=True, skip_group_check=True)
                    emit_k_steps(hp, 1)
                    smt_v = smt[0:jsz].rearrange("p (b c) -> p b c", c=512)
                    nc.scalar.activation(
                        ptt[0:jsz, :, off:off + w], smt_v[:, :, 0:w], AF.Exp,
                        bias=consts[0:jsz, C_BB:C_BB + 1],
                        scale=sc_col(12, jsz))
                    item = ('tail', hp, ptt)
                if len(lags) >= 2:
                    emit_pv(lags.pop(0))
                lags.append(item)
            for it in lags:
                emit_pv(it)
        s_qkv.close()   # frees xall and wq
        s_att.close()   # frees KT/QT/VA8/pt tiles
        pw1 = top.enter_context(tc.tile_pool(name="w1p", bufs=1))
        w1 = pw1.tile([128, DC * DFF], BF16, tag="w1", name="w1")
        nc.gpsimd.dma_start(w1[:], w1_d[:])
        w2 = pw1.tile([128, FT * D], BF16, tag="w2", name="w2")
        nc.gpsimd.dma_start(w2[:], w2_d[:])

        # ---------------- out-projection + LN2 ----------------
        with ExitStack() as sop:
            pop6 = sop.enter_context(tc.tile_pool(name="op6ps", bufs=1,
                                                  space="PSUM"))
            pbb2 = sop.enter_context(tc.tile_pool(name="bb2ps", bufs=2,
                                                  space="PSUM"))
            for et in range(DC):
                for hi in range(2):
                    o_sb, rrow = o_sbs[2 * et + hi]
                    rb = pbb2.tile([64, NQ], F32, tag="bb2", name="rb")
                    nc.tensor.matmul(rb[:], lhsT=ones_row[0:1, 0:64],
                                     rhs=rrow[:], start=True, stop=True)
                    nc.vector.tensor_mul(ONT8[64 * hi:64 * hi + 64, et, :],
                                         o_sb[0:64, :], rb[:])
            ps_os = [pop6.tile([128, NQ], F32, tag=f"op{dt}", name=f"ps_o{dt}")
                     for dt in range(DC)]
            for dt in range(DC):
                nc.tensor.matmul(ps_os[dt][:],
                                 lhsT=bvec[0:1, dt * 128:(dt + 1) * 128],
                                 rhs=ones_rq[:], start=True, stop=False,
                                 skip_group_check=True)
            for pe in range(3):
                for dt in range(DC):
                    nc.tensor.matmul(
                        ps_os[dt][:],
                        lhsT=wout[:, 2 * pe:2 * pe + 2,
                                  dt * 128:(dt + 1) * 128],
                        rhs=ONT8[:, 2 * pe:2 * pe + 2, :],
                        start=False, stop=(pe == 2),
                        perf_mode=DR, skip_group_check=True)
            for dt in range(DC):
                nc.vector.scalar_tensor_tensor(
                    x2T[dt][:], ps_os[dt][:],
                    consts[:, C_SCO:C_SCO + 1],
                    nqT[:, dt * NQ:(dt + 1) * NQ], op0=ALU.mult, op1=ALU.add)

        with ExitStack() as s:
            pst2 = s.enter_context(tc.tile_pool(name="st2ps", bufs=1, space="PSUM"))
            pbc2 = s.enter_context(tc.tile_pool(name="bc2ps", bufs=2, space="PSUM"))
            psq2 = s.enter_context(tc.tile_pool(name="sq2p", bufs=2))
            mu_ps = pst2.tile([1, NQ], F32, tag="mu2", name="mu2_ps")
            msq_ps = pst2.tile([1, NQ], F32, tag="ms2", name="msq2_ps")
            for dt in range(DC):
                nc.tensor.matmul(mu_ps[:], lhsT=ones_colf[:], rhs=x2T[dt][:],
                                 start=(dt == 0), stop=(dt == DC - 1))
                sq = psq2.tile([128, NQ], BF16, tag="sq2", name="sq2")
                nc.scalar.square(sq[:], x2T[dt][:])
                nc.tensor.matmul(msq_ps[:], lhsT=ones_col[:], rhs=sq[:],
                                 start=(dt == 0), stop=(dt == DC - 1))
            mu_bf = prow.tile([1, NQ], BF16, tag="mubf", name="mu2_bf")
            nc.scalar.mul(mu_bf[:], mu_ps[:], 1.0 / D)
            msq = prow.tile([1, NQ], F32, tag="msq", name="msq2")
            nc.scalar.mul(msq[:], msq_ps[:], 1.0 / D)
            var = prow.tile([1, NQ], F32, tag="var", name="var2")
            nc.vector.tensor_mul(var[:], mu_bf[:], mu_bf[:])
            nc.vector.tensor_sub(var[:], msq[:], var[:])
            sd = prow.tile([1, NQ], BF16, tag="sd", name="sd2")
            nc.scalar.activation(sd[:], var[:], AF.Sqrt, bias=eps1[0:1, 0:1])
            rs2_bf = prow.tile([1, NQ], BF16, tag="rsbf", name="rs2_bf")
            with nc.allow_low_precision(reason="bf16 LN2 rsqrt on a 2e-2 "
                                               "tolerance output"):
                nc.vector.reciprocal(rs2_bf[:], sd[:])
            mub2 = pbc2.tile([128, NQ], F32, tag="bc2", name="mub2")
            nc.tensor.matmul(mub2[:], lhsT=ones_row[:], rhs=mu_bf[:],
                             start=True, stop=True)
            sb2 = pbc2.tile([128, NQ], F32, tag="bc2", name="sb2")
            nc.tensor.matmul(sb2[:], lhsT=ones_row[:], rhs=rs2_bf[:],
                             start=True, stop=True)
            for dc in range(DC):
                tmp = psq2.tile([128, NQ], F32, tag="tm2", name="tm2")
                nc.vector.tensor_sub(tmp[:], x2T[dc][:], mub2[:])
                nc.vector.tensor_mul(n2T[dc][:], tmp[:], sb2[:])

        # ---------------- FFN (bf16) ----------------
        with ExitStack() as s:
            pacc = s.enter_context(tc.tile_pool(name="accps", bufs=DC, space="PSUM"))
            pm1 = s.enter_context(tc.tile_pool(name="m1ps", bufs=2, space="PSUM"))
            pff = s.enter_context(tc.tile_pool(name="ffp", bufs=3))
            ps_acc = [pacc.tile([128, NQ], F32, tag="acc", name=f"acc{i}")
                      for i in range(DC)]
            for ft in range(FT):
                ps1 = pm1.tile([128, NQ], F32, tag="m1", name="ps1")
                for dc in range(DC):
                    nc.tensor.matmul(
                        ps1[:],
                        lhsT=w1[:, dc * DFF + ft * 128:dc * DFF + (ft + 1) * 128],
                        rhs=n2T[dc][:], start=(dc == 0), stop=(dc == DC - 1))
                sig = pff.tile([128, NQ], BF16, tag="sig", name="sig")
                nc.scalar.activation(sig[:], ps1[:], AF.Sigmoid,
                                     bias=consts[:, C_CB1 + ft:C_CB1 + ft + 1])
                ffs = pff.tile([128, NQ], BF16, tag="ffs", name="ffs")
                nc.vector.scalar_tensor_tensor(
                    ffs[:], ps1[:], consts[:, C_CB1 + ft:C_CB1 + ft + 1], sig[:],
                    op0=ALU.add, op1=ALU.mult)
                for dt in range(DC):
                    nc.tensor.matmul(
                        ps_acc[dt][:],
                        lhsT=w2[:, ft * D + dt * 128:ft * D + (dt + 1) * 128],
                        rhs=ffs[:], start=(ft == 0), stop=(ft == FT - 1),
                        skip_group_check=True)
            for dt in range(DC):
                nc.vector.scalar_tensor_tensor(
                    outT[dt][:], ps_acc[dt][:], consts[:, C_B2 + dt:C_B2 + dt + 1],
                    x2T[dt][:], op0=ALU.add, op1=ALU.add)

        # ---------------- store (transpose to token-major) ----------------
        with ExitStack() as s:
            ptr2 = s.enter_context(tc.tile_pool(name="trps2", bufs=2, space="PSUM"))
            posb2 = s.enter_context(tc.tile_pool(name="osbp2", bufs=2))
            QSZ = [128, 128, 128, 8]
            for qt in range(4):
                qsz = QSZ[qt]
                osb = posb2.tile([128, D], F32, tag="osb2", name="osb")
                for dt in range(DC):
                    tp = ptr2.tile([128, 128], F32, tag="tp", name="tp")
                    nc.tensor.transpose(tp[0:qsz, :],
                                        outT[dt][:, qt * 128:qt * 128 + qsz],
                                        ident[:])
                    if dt % 2 == 0:
                        nc.scalar.copy(osb[0:qsz, dt * 128:(dt + 1) * 128],
                                       tp[0:qsz, :])
                    else:
                        nc.vector.tensor_copy(osb[0:qsz, dt * 128:(dt + 1) * 128],
                                              tp[0:qsz, :])
                nc.sync.dma_start(out_d[qt * 128:qt * 128 + qsz, :], osb[0:qsz, :])

    nc.finalize()
    return nc


_NC = None


def _get_nc():
    global _NC
    if _NC is None:
        _NC = build_program()
    return _NC


def _stripes(mat, nstripe):
    """[nstripe*128, C] -> [128, nstripe*C] with stripe i at cols [i*C,(i+1)*C)."""
    r, c = mat.shape
    assert r == nstripe * 128
    return np.ascontiguousarray(
        mat.reshape(nstripe, 128, c).transpose(1, 0, 2).reshape(128, nstripe * c))


def _q_rows(hf):
    """Reordered query rows (within x, 0-based): [f4|f5|f6|f7] x 98."""
    return np.concatenate([np.arange(98) + f * NPATCH + hf * 98
                           for f in range(4)])


def _host_prepare(inputs):
    f32 = np.float32
    bf16 = ml_dtypes.bfloat16
    fp8 = ml_dtypes.float8_e4m3
    x = np.asarray(inputs["x"], f32)
    memory = np.asarray(inputs["memory"], f32)
    w_qkv = np.asarray(inputs["w_qkv"], f32)
    w_out = np.asarray(inputs["w_out"], f32)
    b_out = np.asarray(inputs["b_out"], f32)
    g_att = np.asarray(inputs["ln_att_g"], f32)
    b_att = np.asarray(inputs["ln_att_b"], f32)
    g2 = np.asarray(inputs["ln2_g"], f32)
    bb2 = np.asarray(inputs["ln2_b"], f32)
    w1 = np.asarray(inputs["w1"], f32)
    b1 = np.asarray(inputs["b1"], f32)
    w2 = np.asarray(inputs["w2"], f32)
    b2v = np.asarray(inputs["b2"], f32)

    w_qkv_eff = w_qkv * g_att[None, :]
    cb_qkv = w_qkv @ b_att
    cb_q4 = (4.0 * cb_qkv[:D]).astype(f32)
    cb_v = cb_qkv[2 * D:].astype(f32)
    b_out_eff = (b_out + w_out @ cb_v).astype(f32)
    w1_eff = w1 * g2[None, :]
    cb1_eff = (w1 @ bb2 + b1).astype(f32)

    def cols(v):
        return np.ascontiguousarray(v.reshape(-1, 128).T)

    shared = {
        "wq": _stripes(np.ascontiguousarray(4.0 * w_qkv_eff.T), DC).astype(fp8),
        "wout": _stripes(np.ascontiguousarray(4.0 * w_out.T), DC).astype(fp8),
        "w1": _stripes(np.ascontiguousarray(w1_eff.T), DC).astype(bf16),
        "w2": _stripes(np.ascontiguousarray(w2.T), FT).astype(bf16),
        "bvec": np.ascontiguousarray((64.0 * b_out_eff)[None, :]).astype(bf16),
    }
    cc = np.zeros((128, NCONST), f32)
    cc[:, C_CBQ:C_CBQ + DC] = cols(cb_q4)
    cc[:, C_B2:C_B2 + DC] = cols(b2v)
    cc[:, C_CB1:C_CB1 + FT] = cols(cb1_eff)
    cc[:, C_SCO] = 1.0 / 64.0
    cc[:, C_BB] = -4.0
    p = np.arange(128)
    for zc, (b0, _, _) in enumerate([ZEROPASS[7], ZEROPASS[9], ZEROPASS[10]]):
        cc[:, C_ZP + zc] = (p < b0).astype(f32)
    shared["consts"] = cc

    in_maps = []
    for c in range(NCORES):
        b, hf = divmod(c, 2)
        x_aug = np.concatenate([memory[b, :T], x[b]], axis=0)      # [L, D]
        xT = x_aug.T    # [768, 1568] -> chunk-major [128, sum(DC*LCH)]
        xall_np = np.concatenate(
            [xT[dc * 128:(dc + 1) * 128, ci * 512:ci * 512 + LCH[ci]]
             for ci in range(4) for dc in range(DC)], axis=1)
        q = x_aug[T + _q_rows(hf)]                                 # [NQ, D]
        in_maps.append({
            "xall": np.ascontiguousarray(xall_np).astype(fp8),
            "xq": _stripes(np.ascontiguousarray(q.T), DC).astype(bf16),
            **shared,
        })
    return in_maps


def _assemble(results):
    out = np.zeros((B, T, D), np.float32)
    for c in range(NCORES):
        b, hf = divmod(c, 2)
        out[b, _q_rows(hf), :] = results[c]["out"]
    return out


def kernel(**inputs):
    nc = _get_nc()
    in_maps = _host_prepare(inputs)
    res = run_bass_kernel_spmd(nc, in_maps, list(range(NCORES)))
    return _assemble(res.results)


def _ensure_ntff_hook():
    """Provide antenv.axon_hooks (absent in this image) so trace=True can
    drive NTFF capture through libaxon_pjrt.so, mirroring trn_boot.py."""
    import contextlib
    import ctypes
    import types

    try:
        from antenv.axon_hooks import get_axon_ntff_profile_hook  # noqa: F401
        return
    except ImportError:
        pass
    import antenv

    so_path = "/opt/axon/libaxon_pjrt.so"
    lib = ctypes.CDLL(so_path)
    if not hasattr(lib, "axon_start_nrt_profile"):
        raise RuntimeError("libaxon_pjrt.so lacks NTFF profile symbols")
    lib.axon_start_nrt_profile.argtypes = [ctypes.POINTER(ctypes.c_int64),
                                           ctypes.c_size_t]
    lib.axon_start_nrt_profile.restype = ctypes.c_int64
    lib.axon_stop_nrt_profile.argtypes = [ctypes.c_char_p]
    lib.axon_stop_nrt_profile.restype = ctypes.c_int64

    @contextlib.contextmanager
    def _hook(output_dir, device_ids):
        import jax
        jax.devices()
        if device_ids:
            ids = (ctypes.c_int64 * len(device_ids))(*device_ids)
            rc = lib.axon_start_nrt_profile(ids, len(device_ids))
        else:
            rc = lib.axon_start_nrt_profile(None, 0)
        if rc != 0:
            raise RuntimeError(f"axon_start_nrt_profile rc={rc}")
        try:
            yield
        finally:
            n = lib.axon_stop_nrt_profile(str(output_dir).encode())
            print(f"ntff profile: {n} file(s) written to {output_dir}",
                  file=sys.stderr)

    box = {"h": _hook}
    mod = types.ModuleType("antenv.axon_hooks")
    mod.set_axon_ntff_profile_hook = lambda h: box.__setitem__("h", h)
    mod.get_axon_ntff_profile_hook = lambda: box["h"]
    sys.modules["antenv.axon_hooks"] = mod
    antenv.axon_hooks = mod


def kernel_traced(**inputs):
    """Like kernel() but with NTFF profiling; returns (out, exec_time_ns)."""
    import tempfile

    from concourse import bass_utils as _bu
    _ensure_ntff_hook()
    _bu.upload_artifacts = lambda tmpdir: f"local:{tmpdir}"  # no bucket creds here
    nc = _get_nc()
    in_maps = _host_prepare(inputs)
    tmpdir = tempfile.mkdtemp(prefix="ntff_")
    res = run_bass_kernel_spmd(nc, in_maps, list(range(NCORES)), trace=True,
                               tmpdir=tmpdir)
    return _assemble(res.results), res.exec_time_ns


# revision 27
# speedup vs baseline: 1.0094x; 1.0094x over previous
"""Trainium2 Bass kernel: LookupTransformerBlock (block-causal sparse attention).

Reference semantics (B=4, T=784, D=768, H=12, Dh=64, d_ff=3072):
  x_aug = LN1(concat(memory[:, :T], x))              # [B, 2T, D], ln1 g=1/b=0
  h     = LN_att(x_aug)  (== x_aug up to O(eps) since x_aug is normalized)
  qkv   = h @ w_qkv.T ; block-causal attention over frames of 196
  x2    = x_aug + attn_out
  out   = (x2 + FFN(LN2(x2)))[:, T:, :]

Sharding: 8 cores = (batch b in 0..3) x (frame-half hf in 0..1); each core's
392 query rows are rows [hf*98, hf*98+98) of each of the 4 x-frames, ordered
[f4|f5|f6|f7].  K/V over all 1568 positions (data parallel, no collectives).

v3 design (vs bf16 v2 at 291us):
  - fp8e4m3 DoubleRow matmuls (2x PE throughput, HW-verified) for the K, V,
    Q GEMMs, the PV accumulation and the out-projection.  QK scores and the
    FFN stay bf16 (FFN fp8 measured at 3e-2 rel err vs the 2e-2 budget).
  - weight blocks are scaled x4 host-side (w_qkv entries ~N(0,1/768) sit in
    the fp8 subnormal range); the factors fold into existing scale operands:
    Q*K x16 into the exp scale, V x4 into the rs/4 evacuation scalar,
    out-proj (x4 w, x16 ONT) into a 1/64 evac multiply.
  - exp shift -4.0 keeps exp scores inside fp8 range (max |dots| = 8.8).
  - frame-ordered queries make the mask structure uniform across cores:
    score/PV matmuls shrink widths on high j-tiles (19% less QK/PV/exp), and
    the only masking left is three static 98-col zero passes per head pair.
  - the 12 per-head softmax denominators are DMA-gathered into a [12, NQ]
    tile for one 12-lane reciprocal (was 12 single-lane 2us reciprocals).
"""

import os
import sys
from contextlib import ExitStack

import numpy as np
import ml_dtypes

for _p in ("/opt/trn_rl_repo", os.path.expanduser("~/.axon_site/_ro/trn_rl_repo")):
    if os.path.isdir(_p) and _p not in sys.path:
        sys.path.append(_p)

import concourse.bass as bass
import concourse.bacc as bacc
import concourse.mybir as mybir
import concourse.tile as tile
from concourse.bass_utils import run_bass_kernel_spmd
from concourse.masks import make_identity

F32 = mybir.dt.float32
BF16 = mybir.dt.bfloat16
FP8 = mybir.dt.float8e4
DR = mybir.MatmulPerfMode.DoubleRow
AF = mybir.ActivationFunctionType
ALU = mybir.AluOpType

B = 4
T = 784
D = 768
L = 2 * T            # 1568
NQ = 392             # query rows per core
H = 12
DH = 64
DFF = 3072
NPATCH = 196
DC = D // 128        # 6
FT = DFF // 128      # 24
NJT = 13             # j-tiles over L (12 x 128 + 32)
JSZ = [128] * 12 + [32]
LCH = [512, 512, 512, 32]
EPS = 1e-5
NCORES = 8

# query-width tables (frame-ordered queries: cols [f4|f5|f6|f7] x 98)
OFFW = [(0, 392)] * 8 + [(98, 294), (98, 294), (196, 196), (294, 98), (294, 98)]
# jt -> (first masked key within tile, query col of the partial frame,
#        consts column holding the 0/1 key mask)
ZEROPASS = {7: (84, 0, 0), 9: (24, 98, 1), 10: (92, 196, 2)}
PAIRW = [(0, 392), (0, 392), (0, 392), (0, 392), (98, 294), (196, 196)]

# consts column layout
C_CBQ = 0            # 6:  4 * (w_qkv[:D] @ b_att)
C_B2 = 6             # 6:  b2
C_CB1 = 12           # 24: w1_eff bias (x1; FFN is bf16)
C_SCO = 36           # 1:  1/64  (out-proj evac scale)
C_BB = 37            # 1:  -4.0  (exp shift)
C_ZP = 38            # 3:  0/1 key masks for the j-tiles with a frame boundary
NCONST = 41

QK_SCALE = DH ** -0.5
# smt = (4Q)(4K) = 16*dots_unnorm; exp scale must be rs*qscale/16 and the
# rs columns hold rs/4, so the constant factor is qscale/4.
SC_MUL = QK_SCALE / 4.0


def _ln_stats_bf(nc, pst, psq, env, xblocks, lch):
    """bf16 column stats (used for the q-slice): sum and sum-of-squares."""
    ones_col = env["ones_col"]
    mu_ps = pst.tile([1, lch], F32, tag="mu", name="mu_ps")
    msq_ps = pst.tile([1, lch], F32, tag="ms", name="msq_ps")
    for dc in range(DC):
        nc.tensor.matmul(mu_ps[:], lhsT=ones_col[:], rhs=xblocks[dc],
                         start=(dc == 0), stop=(dc == DC - 1))
    for dc in range(DC):
        sq = psq.tile([128, lch], BF16, tag="sq", name="sq")
        if dc % 2 == 0:
            nc.scalar.square(sq[:], xblocks[dc])
        else:
            nc.gpsimd.tensor_mul(sq[:], xblocks[dc], xblocks[dc])
        nc.tensor.matmul(msq_ps[:], lhsT=ones_col[:], rhs=sq[:],
                         start=(dc == 0), stop=(dc == DC - 1))
    return mu_ps, msq_ps


def _ln_stats_fp8(nc, pst, psq, env, x8c, lch):
    """fp8 chunk stats: mu via DoubleRow over dc pairs, msq via bf16 squares."""
    ones_col = env["ones_col"]
    ones8 = env["ones8"]
    mu_ps = pst.tile([1, lch], F32, tag="mu", name="mu_ps")
    msq_ps = pst.tile([1, lch], F32, tag="ms", name="msq_ps")
    for dc in range(DC):
        nc.tensor.matmul(mu_ps[:], lhsT=ones8[:, 0:1], rhs=x8c[:, dc, :],
                         start=(dc == 0), stop=(dc == DC - 1))
    for dc in range(DC):
        sq = psq.tile([128, lch], BF16, tag="sq", name="sq")
        if dc % 2 == 0:
            nc.scalar.square(sq[:], x8c[:, dc, :])
        else:
            nc.gpsimd.tensor_mul(sq[:], x8c[:, dc, :], x8c[:, dc, :])
        nc.tensor.matmul(msq_ps[:], lhsT=ones_col[:], rhs=sq[:],
                         start=(dc == 0), stop=(dc == DC - 1))
    return mu_ps, msq_ps


def _ln_rows(nc, prow, env, mu_ps, msq_ps, lch, do_rcp=True, sd_scale=1.0):
    """mu/sd row math; sd_scale>1 bakes a constant into sd (so downstream
    reciprocals produce rs/sd_scale)."""
    mu_bf = prow.tile([1, lch], BF16, tag="mubf", name="mu_bf")
    nc.scalar.mul(mu_bf[:], mu_ps[:], 1.0 / D)
    msq = prow.tile([1, lch], F32, tag="msq", name="msq")
    nc.scalar.mul(msq[:], msq_ps[:], 1.0 / D)
    var = prow.tile([1, lch], F32, tag="var", name="var")
    nc.gpsimd.tensor_mul(var[:], mu_bf[:], mu_bf[:])
    nc.gpsimd.tensor_sub(var[:], msq[:], var[:])
    sd = prow.tile([1, lch], F32, tag="sd", name="sd")
    s2 = sd_scale * sd_scale
    eps_ap = env["eps16"] if sd_scale == 4.0 else env["eps1"]
    nc.scalar.activation(sd[:], var[:], AF.Sqrt, bias=eps_ap[0:1, 0:1], scale=s2)
    if not do_rcp:
        return mu_bf, sd, None
    rs_t = prow.tile([1, lch], F32, tag="rsf", name="rs_f")
    nc.vector.reciprocal(rs_t[:], sd[:])
    rs_bf = prow.tile([1, lch], BF16, tag="rsbf", name="rs_bf")
    nc.scalar.mul(rs_bf[:], rs_t[:], 1.0)
    return mu_bf, rs_t, rs_bf


def build_program():
    nc = bacc.Bacc("TRN2")
    xall_d = nc.declare_dram_parameter("xall", [128, DC * L], FP8, isOutput=False)
    xq_d = nc.declare_dram_parameter("xq", [128, DC * NQ], BF16, isOutput=False)
    wq_d = nc.declare_dram_parameter("wq", [128, DC * 3 * D], FP8, isOutput=False)
    wout_d = nc.declare_dram_parameter("wout", [128, DC * D], FP8, isOutput=False)
    w1_d = nc.declare_dram_parameter("w1", [128, DC * DFF], BF16, isOutput=False)
    w2_d = nc.declare_dram_parameter("w2", [128, FT * D], BF16, isOutput=False)
    consts_d = nc.declare_dram_parameter("consts", [128, NCONST], F32, isOutput=False)
    bvec_d = nc.declare_dram_parameter("bvec", [1, D], BF16, isOutput=False)
    out_d = nc.declare_dram_parameter("out", [NQ, D], F32, isOutput=True)

    env = {}
    with tile.TileContext(nc) as tc, ExitStack() as top:
        pc = top.enter_context(tc.tile_pool(name="const", bufs=1))
        consts = pc.tile([128, NCONST], F32, tag="consts", name="consts")
        nc.sync.dma_start(consts[:], consts_d[:])
        bvec = pc.tile([1, D], BF16, tag="bvec", name="bvec")
        nc.sync.dma_start(bvec[:], bvec_d[:])
        ones_col = pc.tile([128, 1], BF16, tag="onc", name="ones_col")
        nc.vector.memset(ones_col[:], 1.0)
        ones_colf = pc.tile([128, 1], F32, tag="oncf", name="ones_colf")
        nc.vector.memset(ones_colf[:], 1.0)
        ones_row = pc.tile([1, 128], BF16, tag="onr", name="ones_row")
        nc.vector.memset(ones_row[:], 1.0)
        ones_rowf = pc.tile([1, 128], F32, tag="onrf", name="ones_rowf")
        nc.vector.memset(ones_rowf[:], 1.0)
        ones_rq = pc.tile([1, NQ], BF16, tag="onrq", name="ones_rq")
        nc.vector.memset(ones_rq[:], 1.0)
        ones128 = pc.tile([128, 64], BF16, tag="on128", name="ones128")
        nc.vector.memset(ones128[:], 1.0)
        ones8 = pc.tile([128, 2], FP8, tag="on8", name="ones8")
        nc.vector.memset(ones8[:], 1.0)
        eps1 = pc.tile([1, 1], F32, tag="eps", name="eps1")
        nc.vector.memset(eps1[:], EPS)
        eps16 = pc.tile([1, 1], F32, tag="eps16", name="eps16")
        nc.vector.memset(eps16[:], EPS * 16.0)
        ident = pc.tile([128, 128], F32, tag="ident", name="ident")
        make_identity(nc, ident[:])
        # per-LN-chunk rs/4 columns (token-major) and exp scales
        rsc_c = [pc.tile([128, 4], F32, tag=f"rsc{ci}", name=f"rsc{ci}")
                 for ci in range(4)]
        sc_c = [pc.tile([128, 4], F32, tag=f"sc{ci}", name=f"sc{ci}")
                for ci in range(4)]
        env.update(ones_col=ones_col, ones8=ones8, eps1=eps1, eps16=eps16)

        def rs_col(jt, psz):
            return rsc_c[jt // 4][0:psz, jt % 4:jt % 4 + 1]

        def sc_col(jt, psz):
            return sc_c[jt // 4][0:psz, jt % 4:jt % 4 + 1]

        # chunk-major xall layout: chunk ci holds DC stripes of width LCH[ci]
        XC0 = [0, 3072, 6144, 9216]

        pnq = top.enter_context(tc.tile_pool(name="nqp", bufs=1))
        nqT = pnq.tile([128, DC * NQ], BF16, tag="nq", name="nqT")
        nq8 = pnq.tile([128, DC, NQ], FP8, tag="nq8", name="nq8")
        px2 = top.enter_context(tc.tile_pool(name="x2p", bufs=DC))
        x2T = [px2.tile([128, NQ], F32, tag="x2", name=f"x2T{i}") for i in range(DC)]
        pont = top.enter_context(tc.tile_pool(name="ontp", bufs=1))
        ONT8 = pont.tile([128, DC, NQ], FP8, tag="ont", name="ONT8")
        pn2 = top.enter_context(tc.tile_pool(name="n2p", bufs=DC))
        n2T = [pn2.tile([128, NQ], BF16, tag="n2", name=f"n2T{i}") for i in range(DC)]
        pout = top.enter_context(tc.tile_pool(name="outp", bufs=DC))
        outT = [pout.tile([128, NQ], F32, tag="ot", name=f"outT{i}") for i in range(DC)]
        prow = top.enter_context(tc.tile_pool(name="rows", bufs=1))
        pwA = top.enter_context(tc.tile_pool(name="wAp", bufs=1))
        wout = pwA.tile([128, DC, D], FP8, tag="wo", name="wout")
        posb = top.enter_context(tc.tile_pool(name="osbp", bufs=2 * DC))
        s_att = ExitStack()   # attention-lifetime tiles; freed before w1/w2
        pkt = s_att.enter_context(tc.tile_pool(name="ktp", bufs=DC))
        KT = [pkt.tile([128, L], BF16, tag="kt", name=f"KT{i}") for i in range(DC)]
        pqt = s_att.enter_context(tc.tile_pool(name="qtp", bufs=DC))
        QT = [pqt.tile([128, NQ], BF16, tag="qt", name=f"QT{i}") for i in range(DC)]
        pva = s_att.enter_context(tc.tile_pool(name="vap", bufs=6))
        VA8 = [pva.tile([128, 2, H, 72], FP8, tag="va", name=f"VA8_{i}")
               for i in range(6)]
        pvat = s_att.enter_context(tc.tile_pool(name="vatp", bufs=1))
        VA8t = pvat.tile([32, 1, H, 72], FP8, tag="vat", name="VA8t")
        ppt = s_att.enter_context(tc.tile_pool(name="ptp", bufs=3))
        pptt = s_att.enter_context(tc.tile_pool(name="pttp", bufs=2))

        # short-lived inputs on the right allocator stack (freed mid-program)
        s_qkv = ExitStack()   # xall (rewritten in place to x-mu), wq
        s_ln = ExitStack()    # xq + square scratch; dies after attention setup

        pqkv = s_qkv.enter_context(tc.tile_pool(name="qkvp", bufs=1, side="right"))
        xall = pqkv.tile([128, DC * L], FP8, tag="xa", name="xall")
        wq = pqkv.tile([128, DC, 3 * D], FP8, tag="wq", name="wq")
        pxq = s_ln.enter_context(tc.tile_pool(name="xqp", bufs=1, side="right"))
        xq = pxq.tile([128, DC * NQ], BF16, tag="xq", name="xq")
        for ci in range(3):
            nc.sync.dma_start(xall[:, XC0[ci]:XC0[ci + 1]],
                              xall_d[:, XC0[ci]:XC0[ci + 1]])
        nc.sync.dma_start(xq[:], xq_d[:])
        nc.sync.dma_start(xall[:, XC0[3]:DC * L], xall_d[:, XC0[3]:DC * L])
        nc.sync.dma_start(wq[:], wq_d[:].rearrange("p (k c) -> p k c", k=DC))

        # chunk views: x8c[ci] is [128, DC, lch]
        x8c = [xall[:, XC0[ci]:XC0[ci] + DC * LCH[ci]].rearrange(
            "p (k c) -> p k c", k=DC) for ci in range(4)]

        for lt in range(6):
            nc.gpsimd.memset(VA8[lt][:], 1.0 / 16.0)
        nc.gpsimd.memset(VA8t[:], 1.0 / 16.0)
        nc.gpsimd.dma_start(wout[:], wout_d[:].rearrange("p (k c) -> p k c", k=DC))

        # ---------------- LN1 + Q GEMM ----------------
        xqb = [xq[:, dc * NQ:(dc + 1) * NQ] for dc in range(DC)]

        def emit_chunk_tail(ci, pbc, pst):
            lch = LCH[ci]
            mu_bf, sd, _ = _ln_rows(nc, prow, env, *stq[ci], lch,
                                    do_rcp=False, sd_scale=4.0)
            mub = pbc.tile([128, lch], F32, tag="bc", name="mub")
            nc.tensor.matmul(mub[:], lhsT=ones_row[:], rhs=mu_bf[:],
                             start=True, stop=True)
            for dc in range(DC):
                nc.vector.tensor_sub(x8c[ci][:, dc, :], x8c[ci][:, dc, :], mub[:])
            njc = 4 if ci < 3 else 1
            sdT_ps = pst.tile([128, 4], F32, tag="mu", name="sdT_ps")
            if ci == 3:
                nc.vector.memset(sdT_ps[:], 1.0)
            for k in range(njc):
                cnt = min(128, lch - k * 128)
                nc.tensor.matmul(sdT_ps[0:cnt, k:k + 1],
                                 lhsT=sd[0:1, k * 128:k * 128 + cnt],
                                 rhs=ones_rowf[0:1, 0:1],
                                 start=True, stop=True, skip_group_check=True)
            nc.vector.reciprocal(rsc_c[ci][:], sdT_ps[:])
            nc.scalar.mul(sc_c[ci][:], rsc_c[ci][:], SC_MUL)

        with ExitStack() as s:
            pst = s.enter_context(tc.tile_pool(name="stps", bufs=2, space="PSUM"))
            pbc = s.enter_context(tc.tile_pool(name="bcps", bufs=2, space="PSUM"))
            psv = s.enter_context(tc.tile_pool(name="vps", bufs=1, space="PSUM"))
            psq = s.enter_context(tc.tile_pool(name="sqp", bufs=3, side="right"))

            def emit_kv(ci):
                # K(et=0) for this chunk, then V for its j-tiles (fp8 DR)
                lch = LCH[ci]
                ps_k = pbc.tile([128, lch], F32, tag="bc", name="ps_k")
                for pp in range(3):
                    nc.tensor.matmul(
                        ps_k[:], lhsT=wq[:, 2 * pp:2 * pp + 2, D:D + 128],
                        rhs=x8c[ci][:, 2 * pp:2 * pp + 2, :],
                        start=(pp == 0), stop=(pp == 2), perf_mode=DR)
                nc.scalar.copy(KT[0][:, ci * 512:ci * 512 + lch], ps_k[:])
                for jt in range(4 * ci, min(4 * ci + 4, NJT)):
                    jsz = JSZ[jt]
                    o = (jt % 4) * 128
                    ps_v = psv.tile([128, D], F32, tag="psv", name="ps_v")
                    for pp in range(3):
                        lhsT = x8c[ci][:, 2 * pp:2 * pp + 2, o:o + jsz]
                        nc.tensor.matmul(ps_v[0:jsz, 0:512], lhsT=lhsT,
                                         rhs=wq[:, 2 * pp:2 * pp + 2, 2 * D:2 * D + 512],
                                         start=(pp == 0), stop=(pp == 2),
                                         perf_mode=DR, skip_group_check=True)
                        nc.tensor.matmul(ps_v[0:jsz, 512:D], lhsT=lhsT,
                                         rhs=wq[:, 2 * pp:2 * pp + 2, 2 * D + 512:3 * D],
                                         start=(pp == 0), stop=(pp == 2),
                                         perf_mode=DR, skip_group_check=True)
                    psvv = ps_v[0:jsz, :].rearrange("p (h c) -> p h c", c=64)
                    if jt < 12:
                        dst = VA8[jt // 2][0:jsz, jt % 2, :, 0:64]
                    else:
                        dst = VA8t[0:jsz, 0, :, 0:64]
                    nc.vector.tensor_scalar_mul(dst, psvv, rs_col(jt, jsz))

            stq = [None] * 5
            stq[0] = _ln_stats_fp8(nc, pst, psq, env, x8c[0], LCH[0])
            stq[1] = _ln_stats_fp8(nc, pst, psq, env, x8c[1], LCH[1])
            emit_chunk_tail(0, pbc, pst)
            stq[4] = _ln_stats_bf(nc, pst, psq, env, xqb, NQ)
            emit_chunk_tail(1, pbc, pst)
            stq[2] = _ln_stats_fp8(nc, pst, psq, env, x8c[2], LCH[2])

            # q slice: full normalize (mu and rs)
            mu_bfq, _, rs_bfq = _ln_rows(nc, prow, env, *stq[4], NQ)
            mubq = pbc.tile([128, NQ], F32, tag="bc", name="mubq")
            nc.tensor.matmul(mubq[:], lhsT=ones_row[:], rhs=mu_bfq[:],
                             start=True, stop=True)
            sbq = pbc.tile([128, NQ], F32, tag="bc", name="sbq")
            nc.tensor.matmul(sbq[:], lhsT=ones_row[:], rhs=rs_bfq[:],
                             start=True, stop=True)
            for dc in range(DC):
                tmp = psq.tile([128, NQ], F32, tag="tmq", name="tmq")
                nc.vector.tensor_sub(tmp[:], xqb[dc], mubq[:])
                nc.vector.tensor_mul(nqT[:, dc * NQ:(dc + 1) * NQ], tmp[:], sbq[:])
                nc.scalar.copy(nq8[:, dc, :], nqT[:, dc * NQ:(dc + 1) * NQ])

            # Q GEMM (fp8 DR over dc pairs)
            for et in range(DC):
                ps_q = pbc.tile([128, NQ], F32, tag="bc", name="ps_q")
                for pp in range(3):
                    nc.tensor.matmul(
                        ps_q[:],
                        lhsT=wq[:, 2 * pp:2 * pp + 2, et * 128:(et + 1) * 128],
                        rhs=nq8[:, 2 * pp:2 * pp + 2, :],
                        start=(pp == 0), stop=(pp == 2), perf_mode=DR)
                nc.scalar.activation(QT[et][:], ps_q[:], AF.Identity,
                                     bias=consts[:, C_CBQ + et:C_CBQ + et + 1])

            stq[3] = _ln_stats_fp8(nc, pst, psq, env, x8c[3], LCH[3])
            emit_chunk_tail(2, pbc, pst)
            emit_chunk_tail(3, pbc, pst)
            for ci in range(4):
                emit_kv(ci)
        s_ln.close()

        # ---------------- attention ----------------
        o_sbs = []
        with ExitStack() as s:
            psc = s.enter_context(tc.tile_pool(name="scps", bufs=2, space="PSUM"))
            pso = s.enter_context(tc.tile_pool(name="sops", bufs=1, space="PSUM"))
            pkk = s.enter_context(tc.tile_pool(name="kkps", bufs=2, space="PSUM"))

            kwork = {}  # hp -> list of (et, ci, pp) DR K matmul work items

            def emit_k_steps(hp, n):
                wl = kwork.get(hp)
                for _ in range(n):
                    if not wl:
                        return
                    et, ci, pp = wl.pop(0)
                    lch = LCH[ci]
                    if pp == 0:
                        kwork["ps"] = pkk.tile([128, lch], F32, tag="kk",
                                               name="ps_kk")
                    ps_k = kwork["ps"]
                    nc.tensor.matmul(
                        ps_k[:],
                        lhsT=wq[:, 2 * pp:2 * pp + 2,
                                D + et * 128:D + (et + 1) * 128],
                        rhs=x8c[ci][:, 2 * pp:2 * pp + 2, :],
                        start=(pp == 0), stop=(pp == 2),
                        perf_mode=DR, skip_group_check=True)
                    if pp == 2:
                        nc.vector.tensor_copy(KT[et][:, ci * 512:ci * 512 + lch],
                                              ps_k[:])

            def emit_pv(item):
                kind, hp, o_ps = item[0], item[1], o_ps_by_hp[item[1]]
                if kind == 'pair':
                    p, pt = item[2], item[3]
                    off, w = PAIRW[p]
                    for hi in range(2):
                        h = 2 * hp + hi
                        nc.tensor.matmul(
                            o_ps[hi][0:65, off:off + w],
                            lhsT=VA8[p][:, :, h, 0:65],
                            rhs=pt[:, :, hi, off:off + w],
                            start=(p == 0), stop=False,
                            perf_mode=DR, skip_group_check=True)
                else:
                    ptt = item[2]
                    for hi in range(2):
                        h = 2 * hp + hi
                        nc.tensor.matmul(
                            o_ps[hi][0:65, 294:392],
                            lhsT=VA8t[0:32, 0, h, 0:65],
                            rhs=ptt[0:32, hi, 294:392],
                            start=False, stop=True, skip_group_check=True)
                    for hi in range(2):
                        o_sb = posb.tile([65, NQ], BF16, tag="osb", name="o_sb")
                        nc.vector.tensor_copy(o_sb[:], o_ps[hi][0:65, :])
                        rrow = posb.tile([1, NQ], BF16, tag="rrb", name="rrow")
                        with nc.allow_low_precision(
                                reason="bf16 softmax denominators on a 2e-2 "
                                       "tolerance output"):
                            nc.vector.reciprocal(rrow[:], o_sb[64:65, :])
                        o_sbs.append((o_sb, rrow))

            seq = [(hp, p) for hp in range(DC) for p in range(7)]
            for hp in range(DC - 1):
                kwork[hp] = [(hp + 1, ci, pp) for ci in range(4)
                             for pp in range(3)]
            o_ps_by_hp = {}
            lags = []
            for hp, p in seq:
                if p == 0:
                    o_ps_by_hp[hp] = [
                        pso.tile([128, NQ], F32, tag=f"o{hi}", name=f"o_ps{hi}")
                        for hi in range(2)]
                if p < 6:
                    pt = ppt.tile([128, 2, 2, NQ], FP8, tag="pt", name="pt")
                    if p == 5:
                        nc.gpsimd.memset(pt[:, 1, :, 196:294], 0.0)
                    for sub in range(2):
                        jt = 2 * p + sub
                        jsz = JSZ[jt]
                        off, w = OFFW[jt]
                        smt = psc.tile([128, 1024], F32, tag="smt", name="smt")
                        for hi in range(2):
                            part = 64 * hi
                            nc.tensor.matmul(
                                smt[0:jsz, 512 * hi:512 * hi + w],
                                lhsT=KT[hp][part:part + 64,
                                            jt * 128:jt * 128 + jsz],
                                rhs=QT[hp][part:part + 64, off:off + w],
                                start=True, stop=True, skip_group_check=True)
                        emit_k_steps(hp, 1)
                        smt_v = smt[0:jsz].rearrange("p (b c) -> p b c", c=512)
                        nc.scalar.activation(
                            pt[0:jsz, sub, :, off:off + w],
                            smt_v[:, :, 0:w], AF.Exp,
                            bias=consts[0:jsz, C_BB:C_BB + 1],
                            scale=sc_col(jt, jsz))
                        if jt in ZEROPASS:
                            _, qoff, zc = ZEROPASS[jt]
                            nc.vector.tensor_scalar_mul(
                                pt[0:jsz, sub, :, qoff:qoff + 98],
                                pt[0:jsz, sub, :, qoff:qoff + 98],
                                consts[0:jsz, C_ZP + zc:C_ZP + zc + 1])
                    item = ('pair', hp, p, pt)
                else:
                    ptt = pptt.tile([32, 2, NQ], FP8, tag="ptt", name="ptt")
                    jsz = JSZ[12]
                    off, w = OFFW[12]
                    smt = psc.tile([128, 1024], F32, tag="smt", name="smt")
                    for hi in range(2):
                        part = 64 * hi
                        nc.tensor.matmul(
                            smt[0:jsz, 512 * hi:512 * hi + w],
                            lhsT=KT[hp][part:part + 64, 1536:1536 + jsz],
                            rhs=QT[hp][part:part + 64, off:off + w],
                            start=True, stop=True, skip_group_check=True)
                    emit_k_steps(hp, 1)
                    smt_v = smt[0:jsz].rearrange("p (b c) -> p b c", c=512)
                    nc.scalar.activation(
                        ptt[0:jsz, :, off:off + w], smt_v[:, :, 0:w], AF.Exp,
                        bias=consts[0:jsz, C_BB:C_BB + 1],
                        scale=sc_col(12, jsz))
                    item = ('tail', hp, ptt)
                if len(lags) >= 2:
                    emit_pv(lags.pop(0))
                lags.append(item)
            for it in lags:
                emit_pv(it)
        s_qkv.close()   # frees xall and wq
        s_att.close()   # frees KT/QT/VA8/pt tiles
        pw1 = top.enter_context(tc.tile_pool(name="w1p", bufs=1))
        w1 = pw1.tile([128, DC * DFF], BF16, tag="w1", name="w1")
        nc.gpsimd.dma_start(w1[:], w1_d[:])
        w2 = pw1.tile([128, FT * D], BF16, tag="w2", name="w2")
        nc.gpsimd.dma_start(w2[:], w2_d[:])

        # ---------------- out-projection + LN2 ----------------
        with ExitStack() as sop:
            pop6 = sop.enter_context(tc.tile_pool(name="op6ps", bufs=1,
                                                  space="PSUM"))
            pbb2 = sop.enter_context(tc.tile_pool(name="bb2ps", bufs=2,
                                                  space="PSUM"))
            for et in range(DC):
                for hi in range(2):
                    o_sb, rrow = o_sbs[2 * et + hi]
                    rb = pbb2.tile([64, NQ], F32, tag="bb2", name="rb")
                    nc.tensor.matmul(rb[:], lhsT=ones_row[0:1, 0:64],
                                     rhs=rrow[:], start=True, stop=True)
                    nc.vector.tensor_mul(ONT8[64 * hi:64 * hi + 64, et, :],
                                         o_sb[0:64, :], rb[:])
            ps_os = [pop6.tile([128, NQ], F32, tag=f"op{dt}", name=f"ps_o{dt}")
                     for dt in range(DC)]
            for dt in range(DC):
                nc.tensor.matmul(ps_os[dt][:],
                                 lhsT=bvec[0:1, dt * 128:(dt + 1) * 128],
                                 rhs=ones_rq[:], start=True, stop=False,
                                 skip_group_check=True)
            for pe in range(3):
                for dt in range(DC):
                    nc.tensor.matmul(
                        ps_os[dt][:],
                        lhsT=wout[:, 2 * pe:2 * pe + 2,
                                  dt * 128:(dt + 1) * 128],
                        rhs=ONT8[:, 2 * pe:2 * pe + 2, :],
                        start=False, stop=(pe == 2),
                        perf_mode=DR, skip_group_check=True)
            for dt in range(DC):
                nc.vector.scalar_tensor_tensor(
                    x2T[dt][:], ps_os[dt][:],
                    consts[:, C_SCO:C_SCO + 1],
                    nqT[:, dt * NQ:(dt + 1) * NQ], op0=ALU.mult, op1=ALU.add)

        with ExitStack() as s:
            pst2 = s.enter_context(tc.tile_pool(name="st2ps", bufs=1, space="PSUM"))
            pbc2 = s.enter_context(tc.tile_pool(name="bc2ps", bufs=2, space="PSUM"))
            psq2 = s.enter_context(tc.tile_pool(name="sq2p", bufs=2))
            mu_ps = pst2.tile([1, NQ], F32, tag="mu2", name="mu2_ps")
            msq_ps = pst2.tile([1, NQ], F32, tag="ms2", name="msq2_ps")
            for dt in range(DC):
                nc.tensor.matmul(mu_ps[:], lhsT=ones_colf[:], rhs=x2T[dt][:],
                                 start=(dt == 0), stop=(dt == DC - 1))
                sq = psq2.tile([128, NQ], BF16, tag="sq2", name="sq2")
                nc.scalar.square(sq[:], x2T[dt][:])
                nc.tensor.matmul(msq_ps[:], lhsT=ones_col[:], rhs=sq[:],
                                 start=(dt == 0), stop=(dt == DC - 1))
            mu_bf = prow.tile([1, NQ], BF16, tag="mubf", name="mu2_bf")
            nc.scalar.mul(mu_bf[:], mu_ps[:], 1.0 / D)
            msq = prow.tile([1, NQ], F32, tag="msq", name="msq2")
            nc.scalar.mul(msq[:], msq_ps[:], 1.0 / D)
            var = prow.tile([1, NQ], F32, tag="var", name="var2")
            nc.vector.tensor_mul(var[:], mu_bf[:], mu_bf[:])
            nc.vector.tensor_sub(var[:], msq[:], var[:])
            sd = prow.tile([1, NQ], F32, tag="sd", name="sd2")
            nc.scalar.activation(sd[:], var[:], AF.Sqrt, bias=eps1[0:1, 0:1])
            rs2 = prow.tile([1, NQ], F32, tag="rsf", name="rs2")
            nc.vector.reciprocal(rs2[:], sd[:])
            rs2_bf = prow.tile([1, NQ], BF16, tag="rsbf", name="rs2_bf")
            nc.scalar.mul(rs2_bf[:], rs2[:], 1.0)
            mub2 = pbc2.tile([128, NQ], F32, tag="bc2", name="mub2")
            nc.tensor.matmul(mub2[:], lhsT=ones_row[:], rhs=mu_bf[:],
                             start=True, stop=True)
            sb2 = pbc2.tile([128, NQ], F32, tag="bc2", name="sb2")
            nc.tensor.matmul(sb2[:], lhsT=ones_row[:], rhs=rs2_bf[:],
                             start=True, stop=True)
            for dc in range(DC):
                tmp = psq2.tile([128, NQ], F32, tag="tm2", name="tm2")
                nc.vector.tensor_sub(tmp[:], x2T[dc][:], mub2[:])
                nc.vector.tensor_mul(n2T[dc][:], tmp[:], sb2[:])

        # ---------------- FFN (bf16) ----------------
        with ExitStack() as s:
            pacc = s.enter_context(tc.tile_pool(name="accps", bufs=DC, space="PSUM"))
            pm1 = s.enter_context(tc.tile_pool(name="m1ps", bufs=2, space="PSUM"))
            pff = s.enter_context(tc.tile_pool(name="ffp", bufs=3))
            ps_acc = [pacc.tile([128, NQ], F32, tag="acc", name=f"acc{i}")
                      for i in range(DC)]
            for ft in range(FT):
                ps1 = pm1.tile([128, NQ], F32, tag="m1", name="ps1")
                for dc in range(DC):
                    nc.tensor.matmul(
                        ps1[:],
                        lhsT=w1[:, dc * DFF + ft * 128:dc * DFF + (ft + 1) * 128],
                        rhs=n2T[dc][:], start=(dc == 0), stop=(dc == DC - 1))
                sig = pff.tile([128, NQ], BF16, tag="sig", name="sig")
                nc.scalar.activation(sig[:], ps1[:], AF.Sigmoid,
                                     bias=consts[:, C_CB1 + ft:C_CB1 + ft + 1])
                ffs = pff.tile([128, NQ], BF16, tag="ffs", name="ffs")
                nc.vector.scalar_tensor_tensor(
                    ffs[:], ps1[:], consts[:, C_CB1 + ft:C_CB1 + ft + 1], sig[:],
                    op0=ALU.add, op1=ALU.mult)
                for dt in range(DC):
                    nc.tensor.matmul(
                        ps_acc[dt][:],
                        lhsT=w2[:, ft * D + dt * 128:ft * D + (dt + 1) * 128],
                        rhs=ffs[:], start=(ft == 0), stop=(ft == FT - 1),
                        skip_group_check=True)
            for dt in range(DC):
                nc.vector.scalar_tensor_tensor(
                    outT[dt][:], ps_acc[dt][:], consts[:, C_B2 + dt:C_B2 + dt + 1],
                    x2T[dt][:], op0=ALU.add, op1=ALU.add)

        # ---------------- store (transpose to token-major) ----------------
        with ExitStack() as s:
            ptr2 = s.enter_context(tc.tile_pool(name="trps2", bufs=2, space="PSUM"))
            posb2 = s.enter_context(tc.tile_pool(name="osbp2", bufs=2))
            QSZ = [128, 128, 128, 8]
            for qt in range(4):
                qsz = QSZ[qt]
                osb = posb2.tile([128, D], F32, tag="osb2", name="osb")
                for dt in range(DC):
                    tp = ptr2.tile([128, 128], F32, tag="tp", name="tp")
                    nc.tensor.transpose(tp[0:qsz, :],
                                        outT[dt][:, qt * 128:qt * 128 + qsz],
                                        ident[:])
                    if dt % 2 == 0:
                        nc.scalar.copy(osb[0:qsz, dt * 128:(dt + 1) * 128],
                                       tp[0:qsz, :])
                    else:
                        nc.vector.tensor_copy(osb[0:qsz, dt * 128:(dt + 1) * 128],
                                              tp[0:qsz, :])
                nc.sync.dma_start(out_d[qt * 128:qt * 128 + qsz, :], osb[0:qsz, :])

    nc.finalize()
    return nc


_NC = None


def _get_nc():
    global _NC
    if _NC is None:
        _NC = build_program()
    return _NC


def _stripes(mat, nstripe):
    """[nstripe*128, C] -> [128, nstripe*C] with stripe i at cols [i*C,(i+1)*C)."""
    r, c = mat.shape
    assert r == nstripe * 128
    return np.ascontiguousarray(
        mat.reshape(nstripe, 128, c).transpose(1, 0, 2).reshape(128, nstripe * c))


def _q_rows(hf):
    """Reordered query rows (within x, 0-based): [f4|f5|f6|f7] x 98."""
    return np.concatenate([np.arange(98) + f * NPATCH + hf * 98
                           for f in range(4)])


def _host_prepare(inputs):
    f32 = np.float32
    bf16 = ml_dtypes.bfloat16
    fp8 = ml_dtypes.float8_e4m3
    x = np.asarray(inputs["x"], f32)
    memory = np.asarray(inputs["memory"], f32)
    w_qkv = np.asarray(inputs["w_qkv"], f32)
    w_out = np.asarray(inputs["w_out"], f32)
    b_out = np.asarray(inputs["b_out"], f32)
    g_att = np.asarray(inputs["ln_att_g"], f32)
    b_att = np.asarray(inputs["ln_att_b"], f32)
    g2 = np.asarray(inputs["ln2_g"], f32)
    bb2 = np.asarray(inputs["ln2_b"], f32)
    w1 = np.asarray(inputs["w1"], f32)
    b1 = np.asarray(inputs["b1"], f32)
    w2 = np.asarray(inputs["w2"], f32)
    b2v = np.asarray(inputs["b2"], f32)

    w_qkv_eff = w_qkv * g_att[None, :]
    cb_qkv = w_qkv @ b_att
    cb_q4 = (4.0 * cb_qkv[:D]).astype(f32)
    cb_v = cb_qkv[2 * D:].astype(f32)
    b_out_eff = (b_out + w_out @ cb_v).astype(f32)
    w1_eff = w1 * g2[None, :]
    cb1_eff = (w1 @ bb2 + b1).astype(f32)

    def cols(v):
        return np.ascontiguousarray(v.reshape(-1, 128).T)

    shared = {
        "wq": _stripes(np.ascontiguousarray(4.0 * w_qkv_eff.T), DC).astype(fp8),
        "wout": _stripes(np.ascontiguousarray(4.0 * w_out.T), DC).astype(fp8),
        "w1": _stripes(np.ascontiguousarray(w1_eff.T), DC).astype(bf16),
        "w2": _stripes(np.ascontiguousarray(w2.T), FT).astype(bf16),
        "bvec": np.ascontiguousarray((64.0 * b_out_eff)[None, :]).astype(bf16),
    }
    cc = np.zeros((128, NCONST), f32)
    cc[:, C_CBQ:C_CBQ + DC] = cols(cb_q4)
    cc[:, C_B2:C_B2 + DC] = cols(b2v)
    cc[:, C_CB1:C_CB1 + FT] = cols(cb1_eff)
    cc[:, C_SCO] = 1.0 / 64.0
    cc[:, C_BB] = -4.0
    p = np.arange(128)
    for zc, (b0, _, _) in enumerate([ZEROPASS[7], ZEROPASS[9], ZEROPASS[10]]):
        cc[:, C_ZP + zc] = (p < b0).astype(f32)
    shared["consts"] = cc

    in_maps = []
    for c in range(NCORES):
        b, hf = divmod(c, 2)
        x_aug = np.concatenate([memory[b, :T], x[b]], axis=0)      # [L, D]
        xT = x_aug.T    # [768, 1568] -> chunk-major [128, sum(DC*LCH)]
        xall_np = np.concatenate(
            [xT[dc * 128:(dc + 1) * 128, ci * 512:ci * 512 + LCH[ci]]
             for ci in range(4) for dc in range(DC)], axis=1)
        q = x_aug[T + _q_rows(hf)]                                 # [NQ, D]
        in_maps.append({
            "xall": np.ascontiguousarray(xall_np).astype(fp8),
            "xq": _stripes(np.ascontiguousarray(q.T), DC).astype(bf16),
            **shared,
        })
    return in_maps


def _assemble(results):
    out = np.zeros((B, T, D), np.float32)
    for c in range(NCORES):
        b, hf = divmod(c, 2)
        out[b, _q_rows(hf), :] = results[c]["out"]
    return out


def kernel(**inputs):
    nc = _get_nc()
    in_maps = _host_prepare(inputs)
    res = run_bass_kernel_spmd(nc, in_maps, list(range(NCORES)))
    return _assemble(res.results)


def _ensure_ntff_hook():
    """Provide antenv.axon_hooks (absent in this image) so trace=True can
    drive NTFF capture through libaxon_pjrt.so, mirroring trn_boot.py."""
    import contextlib
    import ctypes
    import types

    try:
        from antenv.axon_hooks import get_axon_ntff_profile_hook  # noqa: F401
        return
    except ImportError:
        pass
    import antenv

    so_path = "/opt/axon/libaxon_pjrt.so"
    lib = ctypes.CDLL(so_path)
    if not hasattr(lib, "axon_start_nrt_profile"):
        raise RuntimeError("libaxon_pjrt.so lacks NTFF profile symbols")
    lib.axon_start_nrt_profile.argtypes = [ctypes.POINTER(ctypes.c_int64),
                                           ctypes.c_size_t]
    lib.axon_start_nrt_profile.restype = ctypes.c_int64
    lib.axon_stop_nrt_profile.argtypes = [ctypes.c_char_p]
    lib.axon_stop_nrt_profile.restype = ctypes.c_int64

    @contextlib.contextmanager
    def _hook(output_dir, device_ids):
        import jax
        jax.devices()
        if device_ids:
            ids = (ctypes.c_int64 * len(device_ids))(*device_ids)
            rc = lib.axon_start_nrt_profile(ids, len(device_ids))
        else:
            rc = lib.axon_start_nrt_profile(None, 0)
        if rc != 0:
            raise RuntimeError(f"axon_start_nrt_profile rc={rc}")
        try:
            yield
        finally:
            n = lib.axon_stop_nrt_profile(str(output_dir).encode())
            print(f"ntff profile: {n} file(s) written to {output_dir}",
                  file=sys.stderr)

    box = {"h": _hook}
    mod = types.ModuleType("antenv.axon_hooks")
    mod.set_axon_ntff_profile_hook = lambda h: box.__setitem__("h", h)
    mod.get_axon_ntff_profile_hook = lambda: box["h"]
    sys.modules["antenv.axon_hooks"] = mod
    antenv.axon_hooks = mod


def kernel_traced(**inputs):
    """Like kernel() but with NTFF profiling; returns (out, exec_time_ns)."""
    import tempfile

    from concourse import bass_utils as _bu
    _ensure_ntff_hook()
    _bu.upload_artifacts = lambda tmpdir: f"local:{tmpdir}"  # no bucket creds here
    nc = _get_nc()
    in_maps = _host_prepare(inputs)
    tmpdir = tempfile.mkdtemp(prefix="ntff_")
    res = run_bass_kernel_spmd(nc, in_maps, list(range(NCORES)), trace=True,
                               tmpdir=tmpdir)
    return _assemble(res.results), res.exec_time_ns


# revision 29
# speedup vs baseline: 1.0520x; 1.0421x over previous
"""Trainium2 Bass kernel: LookupTransformerBlock (block-causal sparse attention).

Reference semantics (B=4, T=784, D=768, H=12, Dh=64, d_ff=3072):
  x_aug = LN1(concat(memory[:, :T], x))              # [B, 2T, D], ln1 g=1/b=0
  h     = LN_att(x_aug)  (== x_aug up to O(eps) since x_aug is normalized)
  qkv   = h @ w_qkv.T ; block-causal attention over frames of 196
  x2    = x_aug + attn_out
  out   = (x2 + FFN(LN2(x2)))[:, T:, :]

Sharding: 8 cores = (batch b in 0..3) x (frame-half hf in 0..1); each core's
392 query rows are rows [hf*98, hf*98+98) of each of the 4 x-frames, ordered
[f4|f5|f6|f7].  K/V over all 1568 positions (data parallel, no collectives).

v3 design (vs bf16 v2 at 291us):
  - fp8e4m3 DoubleRow matmuls (2x PE throughput, HW-verified) for the K, V,
    Q GEMMs, the PV accumulation and the out-projection.  QK scores and the
    FFN stay bf16 (FFN fp8 measured at 3e-2 rel err vs the 2e-2 budget).
  - weight blocks are scaled x4 host-side (w_qkv entries ~N(0,1/768) sit in
    the fp8 subnormal range); the factors fold into existing scale operands:
    Q*K x16 into the exp scale, V x4 into the rs/4 evacuation scalar,
    out-proj (x4 w, x16 ONT) into a 1/64 evac multiply.
  - exp shift -4.0 keeps exp scores inside fp8 range (max |dots| = 8.8).
  - frame-ordered queries make the mask structure uniform across cores:
    score/PV matmuls shrink widths on high j-tiles (19% less QK/PV/exp), and
    the only masking left is three static 98-col zero passes per head pair.

Known constraints found on HW (sim does not catch these):
  - reciprocal_approx_fast mis-executes on this device (garbage rs -> NaN);
    use plain nc.vector.reciprocal.
  - DoubleRow LDWEIGHTS requires the stationary k-pair plane stride to be
    32-byte aligned (VA8 pads each head block 65 -> 72 cols).
  - bf16 transpose-through-PSUM corrupts values; the output store stays f32.
"""

import os
import sys
from contextlib import ExitStack

import numpy as np
import ml_dtypes

for _p in ("/opt/trn_rl_repo", os.path.expanduser("~/.axon_site/_ro/trn_rl_repo")):
    if os.path.isdir(_p) and _p not in sys.path:
        sys.path.append(_p)

import concourse.bass as bass
import concourse.bacc as bacc
import concourse.mybir as mybir
import concourse.tile as tile
from concourse.bass_utils import run_bass_kernel_spmd
from concourse.masks import make_identity

F32 = mybir.dt.float32
BF16 = mybir.dt.bfloat16
FP8 = mybir.dt.float8e4
DR = mybir.MatmulPerfMode.DoubleRow
AF = mybir.ActivationFunctionType
ALU = mybir.AluOpType

B = 4
T = 784
D = 768
L = 2 * T            # 1568
NQ = 392             # query rows per core
H = 12
DH = 64
DFF = 3072
NPATCH = 196
DC = D // 128        # 6
FT = DFF // 128      # 24
NJT = 13             # j-tiles over L (12 x 128 + 32)
JSZ = [128] * 12 + [32]
LCH = [512, 512, 512, 32]
EPS = 1e-5
NCORES = 8

# query-width tables (frame-ordered queries: cols [f4|f5|f6|f7] x 98)
OFFW = [(0, 392)] * 8 + [(98, 294), (98, 294), (196, 196), (294, 98), (294, 98)]
# jt -> (first masked key within tile, query col of the partial frame,
#        consts column holding the 0/1 key mask)
ZEROPASS = {7: (84, 0, 0), 9: (24, 98, 1), 10: (92, 196, 2)}
PAIRW = [(0, 392), (0, 392), (0, 392), (0, 392), (98, 294), (196, 196)]

# consts column layout
C_CBQ = 0            # 6:  4 * (w_qkv[:D] @ b_att)
C_B2 = 6             # 6:  b2
C_CB1 = 12           # 24: w1_eff bias (x1; FFN is bf16)
C_SCO = 36           # 1:  1/64  (out-proj evac scale)
C_BB = 37            # 1:  -4.0  (exp shift)
C_ZP = 38            # 3:  0/1 key masks for the j-tiles with a frame boundary
NCONST = 41

QK_SCALE = DH ** -0.5
# smt = (4Q)(4K) = 16*dots_unnorm; exp scale must be rs*qscale/16 and the
# rs columns hold rs/4, so the constant factor is qscale/4.
SC_MUL = QK_SCALE / 4.0


def _ln_stats_bf(nc, pst, psq, env, xblocks, lch):
    """bf16 column stats (used for the q-slice): sum and sum-of-squares."""
    ones_col = env["ones_col"]
    mu_ps = pst.tile([1, lch], F32, tag="mu", name="mu_ps")
    msq_ps = pst.tile([1, lch], F32, tag="ms", name="msq_ps")
    for dc in range(DC):
        nc.tensor.matmul(mu_ps[:], lhsT=ones_col[:], rhs=xblocks[dc],
                         start=(dc == 0), stop=(dc == DC - 1))
    for dc in range(DC):
        sq = psq.tile([128, lch], BF16, tag="sq", name="sq")
        if dc % 2 == 0:
            nc.scalar.square(sq[:], xblocks[dc])
        else:
            nc.gpsimd.tensor_mul(sq[:], xblocks[dc], xblocks[dc])
        nc.tensor.matmul(msq_ps[:], lhsT=ones_col[:], rhs=sq[:],
                         start=(dc == 0), stop=(dc == DC - 1))
    return mu_ps, msq_ps


def _ln_stats_fp8(nc, pst, psq, env, x8c, lch):
    """fp8 chunk stats: mu via DoubleRow over dc pairs, msq via bf16 squares."""
    ones_col = env["ones_col"]
    ones8 = env["ones8"]
    mu_ps = pst.tile([1, lch], F32, tag="mu", name="mu_ps")
    msq_ps = pst.tile([1, lch], F32, tag="ms", name="msq_ps")
    for dc in range(DC):
        nc.tensor.matmul(mu_ps[:], lhsT=ones8[:, 0:1], rhs=x8c[:, dc, :],
                         start=(dc == 0), stop=(dc == DC - 1))
    for dc in range(DC):
        sq = psq.tile([128, lch], BF16, tag="sq", name="sq")
        if dc % 2 == 0:
            nc.scalar.square(sq[:], x8c[:, dc, :])
        else:
            nc.gpsimd.tensor_mul(sq[:], x8c[:, dc, :], x8c[:, dc, :])
        nc.tensor.matmul(msq_ps[:], lhsT=ones_col[:], rhs=sq[:],
                         start=(dc == 0), stop=(dc == DC - 1))
    return mu_ps, msq_ps


def _ln_rows(nc, prow, env, mu_ps, msq_ps, lch, do_rcp=True, sd_scale=1.0):
    """mu/sd row math; sd_scale>1 bakes a constant into sd (so downstream
    reciprocals produce rs/sd_scale)."""
    mu_bf = prow.tile([1, lch], BF16, tag="mubf", name="mu_bf")
    nc.scalar.mul(mu_bf[:], mu_ps[:], 1.0 / D)
    msq = prow.tile([1, lch], F32, tag="msq", name="msq")
    nc.scalar.mul(msq[:], msq_ps[:], 1.0 / D)
    var = prow.tile([1, lch], F32, tag="var", name="var")
    nc.gpsimd.tensor_mul(var[:], mu_bf[:], mu_bf[:])
    nc.gpsimd.tensor_sub(var[:], msq[:], var[:])
    sd = prow.tile([1, lch], F32, tag="sd", name="sd")
    s2 = sd_scale * sd_scale
    eps_ap = env["eps16"] if sd_scale == 4.0 else env["eps1"]
    nc.scalar.activation(sd[:], var[:], AF.Sqrt, bias=eps_ap[0:1, 0:1], scale=s2)
    if not do_rcp:
        return mu_bf, sd, None
    rs_t = prow.tile([1, lch], F32, tag="rsf", name="rs_f")
    nc.vector.reciprocal(rs_t[:], sd[:])
    rs_bf = prow.tile([1, lch], BF16, tag="rsbf", name="rs_bf")
    nc.scalar.mul(rs_bf[:], rs_t[:], 1.0)
    return mu_bf, rs_t, rs_bf


def build_program():
    nc = bacc.Bacc("TRN2")
    xall_d = nc.declare_dram_parameter("xall", [128, DC * L], FP8, isOutput=False)
    xq_d = nc.declare_dram_parameter("xq", [128, DC * NQ], BF16, isOutput=False)
    wq_d = nc.declare_dram_parameter("wq", [128, DC * 3 * D], FP8, isOutput=False)
    wout_d = nc.declare_dram_parameter("wout", [128, DC * D], FP8, isOutput=False)
    w1_d = nc.declare_dram_parameter("w1", [128, DC * DFF], BF16, isOutput=False)
    w2_d = nc.declare_dram_parameter("w2", [128, FT * D], BF16, isOutput=False)
    consts_d = nc.declare_dram_parameter("consts", [128, NCONST], F32, isOutput=False)
    bvec_d = nc.declare_dram_parameter("bvec", [1, D], BF16, isOutput=False)
    out_d = nc.declare_dram_parameter("out", [NQ, D], F32, isOutput=True)

    env = {}
    with tile.TileContext(nc) as tc, ExitStack() as top:
        pc = top.enter_context(tc.tile_pool(name="const", bufs=1))
        consts = pc.tile([128, NCONST], F32, tag="consts", name="consts")
        nc.sync.dma_start(consts[:], consts_d[:])
        bvec = pc.tile([1, D], BF16, tag="bvec", name="bvec")
        nc.sync.dma_start(bvec[:], bvec_d[:])
        ones_col = pc.tile([128, 1], BF16, tag="onc", name="ones_col")
        nc.vector.memset(ones_col[:], 1.0)
        ones_colf = pc.tile([128, 1], F32, tag="oncf", name="ones_colf")
        nc.vector.memset(ones_colf[:], 1.0)
        ones_row = pc.tile([1, 128], BF16, tag="onr", name="ones_row")
        nc.vector.memset(ones_row[:], 1.0)
        ones_rowf = pc.tile([1, 128], F32, tag="onrf", name="ones_rowf")
        nc.vector.memset(ones_rowf[:], 1.0)
        ones_rq = pc.tile([1, NQ], BF16, tag="onrq", name="ones_rq")
        nc.vector.memset(ones_rq[:], 1.0)
        ones128 = pc.tile([128, 64], BF16, tag="on128", name="ones128")
        nc.vector.memset(ones128[:], 1.0)
        ones8 = pc.tile([128, 2], FP8, tag="on8", name="ones8")
        nc.vector.memset(ones8[:], 1.0)
        eps1 = pc.tile([1, 1], F32, tag="eps", name="eps1")
        nc.vector.memset(eps1[:], EPS)
        eps16 = pc.tile([1, 1], F32, tag="eps16", name="eps16")
        nc.vector.memset(eps16[:], EPS * 16.0)
        ident = pc.tile([128, 128], F32, tag="ident", name="ident")
        make_identity(nc, ident[:])
        # per-LN-chunk rs/4 columns (token-major) and exp scales
        rsc_c = [pc.tile([128, 4], F32, tag=f"rsc{ci}", name=f"rsc{ci}")
                 for ci in range(4)]
        sc_c = [pc.tile([128, 4], F32, tag=f"sc{ci}", name=f"sc{ci}")
                for ci in range(4)]
        env.update(ones_col=ones_col, ones8=ones8, eps1=eps1, eps16=eps16)

        def rs_col(jt, psz):
            return rsc_c[jt // 4][0:psz, jt % 4:jt % 4 + 1]

        def sc_col(jt, psz):
            return sc_c[jt // 4][0:psz, jt % 4:jt % 4 + 1]

        # chunk-major xall layout: chunk ci holds DC stripes of width LCH[ci]
        XC0 = [0, 3072, 6144, 9216]

        pnq = top.enter_context(tc.tile_pool(name="nqp", bufs=1))
        nqT = pnq.tile([128, DC * NQ], BF16, tag="nq", name="nqT")
        nq8 = pnq.tile([128, DC, NQ], FP8, tag="nq8", name="nq8")
        px2 = top.enter_context(tc.tile_pool(name="x2p", bufs=DC))
        x2T = [px2.tile([128, NQ], F32, tag="x2", name=f"x2T{i}") for i in range(DC)]
        pont = top.enter_context(tc.tile_pool(name="ontp", bufs=1))
        ONT8 = pont.tile([128, DC, NQ], FP8, tag="ont", name="ONT8")
        pn2 = top.enter_context(tc.tile_pool(name="n2p", bufs=DC))
        n2T = [pn2.tile([128, NQ], BF16, tag="n2", name=f"n2T{i}") for i in range(DC)]
        pout = top.enter_context(tc.tile_pool(name="outp", bufs=DC))
        outT = [pout.tile([128, NQ], F32, tag="ot", name=f"outT{i}") for i in range(DC)]
        prow = top.enter_context(tc.tile_pool(name="rows", bufs=1))
        pwA = top.enter_context(tc.tile_pool(name="wAp", bufs=1))
        wout = pwA.tile([128, DC, D], FP8, tag="wo", name="wout")
        posb = top.enter_context(tc.tile_pool(name="osbp", bufs=2 * DC))
        s_att = ExitStack()   # attention-lifetime tiles; freed before w1/w2
        pkt = s_att.enter_context(tc.tile_pool(name="ktp", bufs=DC))
        KT = [pkt.tile([128, L], BF16, tag="kt", name=f"KT{i}") for i in range(DC)]
        pqt = s_att.enter_context(tc.tile_pool(name="qtp", bufs=DC))
        QT = [pqt.tile([128, NQ], BF16, tag="qt", name=f"QT{i}") for i in range(DC)]
        pva = s_att.enter_context(tc.tile_pool(name="vap", bufs=6))
        VA8 = [pva.tile([128, 2, H, 72], FP8, tag="va", name=f"VA8_{i}")
               for i in range(6)]
        pvat = s_att.enter_context(tc.tile_pool(name="vatp", bufs=1))
        VA8t = pvat.tile([32, 1, H, 72], FP8, tag="vat", name="VA8t")
        ppt = s_att.enter_context(tc.tile_pool(name="ptp", bufs=3))
        pptt = s_att.enter_context(tc.tile_pool(name="pttp", bufs=2))

        # short-lived inputs on the right allocator stack (freed mid-program)
        s_qkv = ExitStack()   # xall (rewritten in place to x-mu), wq
        s_ln = ExitStack()    # xq + square scratch; dies after attention setup

        pqkv = s_qkv.enter_context(tc.tile_pool(name="qkvp", bufs=1, side="right"))
        xall = pqkv.tile([128, DC * L], FP8, tag="xa", name="xall")
        wq = pqkv.tile([128, DC, 3 * D], FP8, tag="wq", name="wq")
        pxq = s_ln.enter_context(tc.tile_pool(name="xqp", bufs=1, side="right"))
        xq = pxq.tile([128, DC * NQ], BF16, tag="xq", name="xq")
        for ci in range(3):
            nc.sync.dma_start(xall[:, XC0[ci]:XC0[ci + 1]],
                              xall_d[:, XC0[ci]:XC0[ci + 1]])
        nc.sync.dma_start(xq[:], xq_d[:])
        nc.sync.dma_start(xall[:, XC0[3]:DC * L], xall_d[:, XC0[3]:DC * L])
        nc.sync.dma_start(wq[:], wq_d[:].rearrange("p (k c) -> p k c", k=DC))

        # chunk views: x8c[ci] is [128, DC, lch]
        x8c = [xall[:, XC0[ci]:XC0[ci] + DC * LCH[ci]].rearrange(
            "p (k c) -> p k c", k=DC) for ci in range(4)]

        for lt in range(6):
            nc.gpsimd.memset(VA8[lt][:], 1.0 / 16.0)
        nc.gpsimd.memset(VA8t[:], 1.0 / 16.0)
        nc.gpsimd.dma_start(wout[:], wout_d[:].rearrange("p (k c) -> p k c", k=DC))

        # ---------------- LN1 + Q GEMM ----------------
        xqb = [xq[:, dc * NQ:(dc + 1) * NQ] for dc in range(DC)]

        def emit_chunk_tail(ci, pbc, pst):
            lch = LCH[ci]
            mu_bf, sd, _ = _ln_rows(nc, prow, env, *stq[ci], lch,
                                    do_rcp=False, sd_scale=4.0)
            mub = pbc.tile([128, lch], F32, tag="bc", name="mub")
            nc.tensor.matmul(mub[:], lhsT=ones_row[:], rhs=mu_bf[:],
                             start=True, stop=True)
            for dc in range(DC):
                nc.vector.tensor_sub(x8c[ci][:, dc, :], x8c[ci][:, dc, :], mub[:])
            njc = 4 if ci < 3 else 1
            sdT_ps = pst.tile([128, 4], F32, tag="mu", name="sdT_ps")
            if ci == 3:
                nc.vector.memset(sdT_ps[:], 1.0)
            for k in range(njc):
                cnt = min(128, lch - k * 128)
                nc.tensor.matmul(sdT_ps[0:cnt, k:k + 1],
                                 lhsT=sd[0:1, k * 128:k * 128 + cnt],
                                 rhs=ones_rowf[0:1, 0:1],
                                 start=True, stop=True, skip_group_check=True)
            nc.vector.reciprocal(rsc_c[ci][:], sdT_ps[:])
            nc.scalar.mul(sc_c[ci][:], rsc_c[ci][:], SC_MUL)

        with ExitStack() as s:
            pst = s.enter_context(tc.tile_pool(name="stps", bufs=2, space="PSUM"))
            pbc = s.enter_context(tc.tile_pool(name="bcps", bufs=2, space="PSUM"))
            psv = s.enter_context(tc.tile_pool(name="vps", bufs=1, space="PSUM"))
            psq = s.enter_context(tc.tile_pool(name="sqp", bufs=3, side="right"))

            def emit_kv(ci):
                # K (all six et blocks) for this chunk, then V for its
                # j-tiles (fp8 DR).  All K lands before attention starts;
                # the setup-phase PE has the slack for it and it frees the
                # PSUM banks attention needs for double-buffered PV outputs.
                lch = LCH[ci]
                for et in range(DC):
                    ps_k = pbc.tile([128, lch], F32, tag="bc", name="ps_k")
                    for pp in range(3):
                        nc.tensor.matmul(
                            ps_k[:],
                            lhsT=wq[:, 2 * pp:2 * pp + 2,
                                    D + et * 128:D + (et + 1) * 128],
                            rhs=x8c[ci][:, 2 * pp:2 * pp + 2, :],
                            start=(pp == 0), stop=(pp == 2), perf_mode=DR)
                    if et % 2 == 0:
                        nc.scalar.copy(KT[et][:, ci * 512:ci * 512 + lch],
                                       ps_k[:])
                    else:
                        nc.vector.tensor_copy(
                            KT[et][:, ci * 512:ci * 512 + lch], ps_k[:])
                for jt in range(4 * ci, min(4 * ci + 4, NJT)):
                    jsz = JSZ[jt]
                    o = (jt % 4) * 128
                    ps_v = psv.tile([128, D], F32, tag="psv", name="ps_v")
                    for pp in range(3):
                        lhsT = x8c[ci][:, 2 * pp:2 * pp + 2, o:o + jsz]
                        nc.tensor.matmul(ps_v[0:jsz, 0:512], lhsT=lhsT,
                                         rhs=wq[:, 2 * pp:2 * pp + 2, 2 * D:2 * D + 512],
                                         start=(pp == 0), stop=(pp == 2),
                                         perf_mode=DR, skip_group_check=True)
                        nc.tensor.matmul(ps_v[0:jsz, 512:D], lhsT=lhsT,
                                         rhs=wq[:, 2 * pp:2 * pp + 2, 2 * D + 512:3 * D],
                                         start=(pp == 0), stop=(pp == 2),
                                         perf_mode=DR, skip_group_check=True)
                    psvv = ps_v[0:jsz, :].rearrange("p (h c) -> p h c", c=64)
                    if jt < 12:
                        dst = VA8[jt // 2][0:jsz, jt % 2, :, 0:64]
                    else:
                        dst = VA8t[0:jsz, 0, :, 0:64]
                    nc.vector.tensor_scalar_mul(dst, psvv, rs_col(jt, jsz))

            stq = [None] * 5
            stq[0] = _ln_stats_fp8(nc, pst, psq, env, x8c[0], LCH[0])
            stq[1] = _ln_stats_fp8(nc, pst, psq, env, x8c[1], LCH[1])
            emit_chunk_tail(0, pbc, pst)
            stq[4] = _ln_stats_bf(nc, pst, psq, env, xqb, NQ)
            emit_chunk_tail(1, pbc, pst)
            stq[2] = _ln_stats_fp8(nc, pst, psq, env, x8c[2], LCH[2])

            # q slice: full normalize (mu and rs)
            mu_bfq, _, rs_bfq = _ln_rows(nc, prow, env, *stq[4], NQ)
            mubq = pbc.tile([128, NQ], F32, tag="bc", name="mubq")
            nc.tensor.matmul(mubq[:], lhsT=ones_row[:], rhs=mu_bfq[:],
                             start=True, stop=True)
            sbq = pbc.tile([128, NQ], F32, tag="bc", name="sbq")
            nc.tensor.matmul(sbq[:], lhsT=ones_row[:], rhs=rs_bfq[:],
                             start=True, stop=True)
            for dc in range(DC):
                tmp = psq.tile([128, NQ], F32, tag="tmq", name="tmq")
                nc.vector.tensor_sub(tmp[:], xqb[dc], mubq[:])
                nc.vector.tensor_mul(nqT[:, dc * NQ:(dc + 1) * NQ], tmp[:], sbq[:])
                nc.scalar.copy(nq8[:, dc, :], nqT[:, dc * NQ:(dc + 1) * NQ])

            # Q GEMM (fp8 DR over dc pairs)
            for et in range(DC):
                ps_q = pbc.tile([128, NQ], F32, tag="bc", name="ps_q")
                for pp in range(3):
                    nc.tensor.matmul(
                        ps_q[:],
                        lhsT=wq[:, 2 * pp:2 * pp + 2, et * 128:(et + 1) * 128],
                        rhs=nq8[:, 2 * pp:2 * pp + 2, :],
                        start=(pp == 0), stop=(pp == 2), perf_mode=DR)
                nc.scalar.activation(QT[et][:], ps_q[:], AF.Identity,
                                     bias=consts[:, C_CBQ + et:C_CBQ + et + 1])

            stq[3] = _ln_stats_fp8(nc, pst, psq, env, x8c[3], LCH[3])
            emit_chunk_tail(2, pbc, pst)
            emit_chunk_tail(3, pbc, pst)
            for ci in range(4):
                emit_kv(ci)
        s_ln.close()

        # ---------------- attention ----------------
        o_sbs = []
        with ExitStack() as s:
            psc = s.enter_context(tc.tile_pool(name="scps", bufs=2, space="PSUM"))
            pso = s.enter_context(tc.tile_pool(name="sops", bufs=2, space="PSUM"))

            def emit_pv(item):
                kind, hp, o_ps = item[0], item[1], o_ps_by_hp[item[1]]
                if kind == 'pair':
                    p, pt = item[2], item[3]
                    off, w = PAIRW[p]
                    for hi in range(2):
                        h = 2 * hp + hi
                        nc.tensor.matmul(
                            o_ps[hi][0:65, off:off + w],
                            lhsT=VA8[p][:, :, h, 0:65],
                            rhs=pt[:, :, hi, off:off + w],
                            start=(p == 0), stop=False,
                            perf_mode=DR, skip_group_check=True)
                else:
                    ptt = item[2]
                    for hi in range(2):
                        h = 2 * hp + hi
                        nc.tensor.matmul(
                            o_ps[hi][0:65, 294:392],
                            lhsT=VA8t[0:32, 0, h, 0:65],
                            rhs=ptt[0:32, hi, 294:392],
                            start=False, stop=True, skip_group_check=True)
                    for hi in range(2):
                        o_sb = posb.tile([65, NQ], BF16, tag="osb", name="o_sb")
                        nc.vector.tensor_copy(o_sb[:], o_ps[hi][0:65, :])
                        rrow = posb.tile([1, NQ], BF16, tag="rrb", name="rrow")
                        with nc.allow_low_precision(
                                reason="bf16 softmax denominators on a 2e-2 "
                                       "tolerance output"):
                            nc.vector.reciprocal(rrow[:], o_sb[64:65, :])
                        o_sbs.append((o_sb, rrow))

            seq = [(hp, p) for hp in range(DC) for p in range(7)]
            o_ps_by_hp = {}
            lags = []
            for hp, p in seq:
                if p == 0:
                    o_ps_by_hp[hp] = [
                        pso.tile([128, NQ], F32, tag=f"o{hi}", name=f"o_ps{hi}")
                        for hi in range(2)]
                if p < 6:
                    pt = ppt.tile([128, 2, 2, NQ], FP8, tag="pt", name="pt")
                    if p == 5:
                        nc.gpsimd.memset(pt[:, 1, :, 196:294], 0.0)
                    for sub in range(2):
                        jt = 2 * p + sub
                        jsz = JSZ[jt]
                        off, w = OFFW[jt]
                        smt = psc.tile([128, 1024], F32, tag="smt", name="smt")
                        for hi in range(2):
                            part = 64 * hi
                            nc.tensor.matmul(
                                smt[0:jsz, 512 * hi:512 * hi + w],
                                lhsT=KT[hp][part:part + 64,
                                            jt * 128:jt * 128 + jsz],
                                rhs=QT[hp][part:part + 64, off:off + w],
                                start=True, stop=True, skip_group_check=True)
                        smt_v = smt[0:jsz].rearrange("p (b c) -> p b c", c=512)
                        nc.scalar.activation(
                            pt[0:jsz, sub, :, off:off + w],
                            smt_v[:, :, 0:w], AF.Exp,
                            bias=consts[0:jsz, C_BB:C_BB + 1],
                            scale=sc_col(jt, jsz))
                        if jt in ZEROPASS:
                            _, qoff, zc = ZEROPASS[jt]
                            nc.vector.tensor_scalar_mul(
                                pt[0:jsz, sub, :, qoff:qoff + 98],
                                pt[0:jsz, sub, :, qoff:qoff + 98],
                                consts[0:jsz, C_ZP + zc:C_ZP + zc + 1])
                    item = ('pair', hp, p, pt)
                else:
                    ptt = pptt.tile([32, 2, NQ], FP8, tag="ptt", name="ptt")
                    jsz = JSZ[12]
                    off, w = OFFW[12]
                    smt = psc.tile([128, 1024], F32, tag="smt", name="smt")
                    for hi in range(2):
                        part = 64 * hi
                        nc.tensor.matmul(
                            smt[0:jsz, 512 * hi:512 * hi + w],
                            lhsT=KT[hp][part:part + 64, 1536:1536 + jsz],
                            rhs=QT[hp][part:part + 64, off:off + w],
                            start=True, stop=True, skip_group_check=True)
                    smt_v = smt[0:jsz].rearrange("p (b c) -> p b c", c=512)
                    nc.scalar.activation(
                        ptt[0:jsz, :, off:off + w], smt_v[:, :, 0:w], AF.Exp,
                        bias=consts[0:jsz, C_BB:C_BB + 1],
                        scale=sc_col(12, jsz))
                    item = ('tail', hp, ptt)
                if len(lags) >= 2:
                    emit_pv(lags.pop(0))
                lags.append(item)
            for it in lags:
                emit_pv(it)
        s_qkv.close()   # frees xall and wq
        s_att.close()   # frees KT/QT/VA8/pt tiles
        pw1 = top.enter_context(tc.tile_pool(name="w1p", bufs=1))
        w1 = pw1.tile([128, DC * DFF], BF16, tag="w1", name="w1")
        nc.gpsimd.dma_start(w1[:], w1_d[:])
        w2 = pw1.tile([128, FT * D], BF16, tag="w2", name="w2")
        nc.gpsimd.dma_start(w2[:], w2_d[:])

        # ---------------- out-projection + LN2 ----------------
        with ExitStack() as sop:
            pop6 = sop.enter_context(tc.tile_pool(name="op6ps", bufs=1,
                                                  space="PSUM"))
            pbb2 = sop.enter_context(tc.tile_pool(name="bb2ps", bufs=2,
                                                  space="PSUM"))
            for et in range(DC):
                for hi in range(2):
                    o_sb, rrow = o_sbs[2 * et + hi]
                    rb = pbb2.tile([64, NQ], F32, tag="bb2", name="rb")
                    nc.tensor.matmul(rb[:], lhsT=ones_row[0:1, 0:64],
                                     rhs=rrow[:], start=True, stop=True)
                    nc.vector.tensor_mul(ONT8[64 * hi:64 * hi + 64, et, :],
                                         o_sb[0:64, :], rb[:])
            ps_os = [pop6.tile([128, NQ], F32, tag=f"op{dt}", name=f"ps_o{dt}")
                     for dt in range(DC)]
            for dt in range(DC):
                nc.tensor.matmul(ps_os[dt][:],
                                 lhsT=bvec[0:1, dt * 128:(dt + 1) * 128],
                                 rhs=ones_rq[:], start=True, stop=False,
                                 skip_group_check=True)
            for pe in range(3):
                for dt in range(DC):
                    nc.tensor.matmul(
                        ps_os[dt][:],
                        lhsT=wout[:, 2 * pe:2 * pe + 2,
                                  dt * 128:(dt + 1) * 128],
                        rhs=ONT8[:, 2 * pe:2 * pe + 2, :],
                        start=False, stop=(pe == 2),
                        perf_mode=DR, skip_group_check=True)
            for dt in range(DC):
                nc.vector.scalar_tensor_tensor(
                    x2T[dt][:], ps_os[dt][:],
                    consts[:, C_SCO:C_SCO + 1],
                    nqT[:, dt * NQ:(dt + 1) * NQ], op0=ALU.mult, op1=ALU.add)

        with ExitStack() as s:
            pst2 = s.enter_context(tc.tile_pool(name="st2ps", bufs=1, space="PSUM"))
            pbc2 = s.enter_context(tc.tile_pool(name="bc2ps", bufs=2, space="PSUM"))
            psq2 = s.enter_context(tc.tile_pool(name="sq2p", bufs=2))
            mu_ps = pst2.tile([1, NQ], F32, tag="mu2", name="mu2_ps")
            msq_ps = pst2.tile([1, NQ], F32, tag="ms2", name="msq2_ps")
            for dt in range(DC):
                nc.tensor.matmul(mu_ps[:], lhsT=ones_colf[:], rhs=x2T[dt][:],
                                 start=(dt == 0), stop=(dt == DC - 1))
                sq = psq2.tile([128, NQ], BF16, tag="sq2", name="sq2")
                nc.scalar.square(sq[:], x2T[dt][:])
                nc.tensor.matmul(msq_ps[:], lhsT=ones_col[:], rhs=sq[:],
                                 start=(dt == 0), stop=(dt == DC - 1))
            mu_bf = prow.tile([1, NQ], BF16, tag="mubf", name="mu2_bf")
            nc.scalar.mul(mu_bf[:], mu_ps[:], 1.0 / D)
            msq = prow.tile([1, NQ], F32, tag="msq", name="msq2")
            nc.scalar.mul(msq[:], msq_ps[:], 1.0 / D)
            var = prow.tile([1, NQ], F32, tag="var", name="var2")
            nc.vector.tensor_mul(var[:], mu_bf[:], mu_bf[:])
            nc.vector.tensor_sub(var[:], msq[:], var[:])
            sd = prow.tile([1, NQ], F32, tag="sd", name="sd2")
            nc.scalar.activation(sd[:], var[:], AF.Sqrt, bias=eps1[0:1, 0:1])
            rs2 = prow.tile([1, NQ], F32, tag="rsf", name="rs2")
            nc.vector.reciprocal(rs2[:], sd[:])
            rs2_bf = prow.tile([1, NQ], BF16, tag="rsbf", name="rs2_bf")
            nc.scalar.mul(rs2_bf[:], rs2[:], 1.0)
            mub2 = pbc2.tile([128, NQ], F32, tag="bc2", name="mub2")
            nc.tensor.matmul(mub2[:], lhsT=ones_row[:], rhs=mu_bf[:],
                             start=True, stop=True)
            sb2 = pbc2.tile([128, NQ], F32, tag="bc2", name="sb2")
            nc.tensor.matmul(sb2[:], lhsT=ones_row[:], rhs=rs2_bf[:],
                             start=True, stop=True)
            for dc in range(DC):
                tmp = psq2.tile([128, NQ], F32, tag="tm2", name="tm2")
                nc.vector.tensor_sub(tmp[:], x2T[dc][:], mub2[:])
                nc.vector.tensor_mul(n2T[dc][:], tmp[:], sb2[:])

        # ---------------- FFN (bf16) ----------------
        with ExitStack() as s:
            pacc = s.enter_context(tc.tile_pool(name="accps", bufs=DC, space="PSUM"))
            pm1 = s.enter_context(tc.tile_pool(name="m1ps", bufs=2, space="PSUM"))
            pff = s.enter_context(tc.tile_pool(name="ffp", bufs=3))
            ps_acc = [pacc.tile([128, NQ], F32, tag="acc", name=f"acc{i}")
                      for i in range(DC)]
            for ft in range(FT):
                ps1 = pm1.tile([128, NQ], F32, tag="m1", name="ps1")
                for dc in range(DC):
                    nc.tensor.matmul(
                        ps1[:],
                        lhsT=w1[:, dc * DFF + ft * 128:dc * DFF + (ft + 1) * 128],
                        rhs=n2T[dc][:], start=(dc == 0), stop=(dc == DC - 1))
                sig = pff.tile([128, NQ], BF16, tag="sig", name="sig")
                nc.scalar.activation(sig[:], ps1[:], AF.Sigmoid,
                                     bias=consts[:, C_CB1 + ft:C_CB1 + ft + 1])
                ffs = pff.tile([128, NQ], BF16, tag="ffs", name="ffs")
                nc.vector.scalar_tensor_tensor(
                    ffs[:], ps1[:], consts[:, C_CB1 + ft:C_CB1 + ft + 1], sig[:],
                    op0=ALU.add, op1=ALU.mult)
                for dt in range(DC):
                    nc.tensor.matmul(
                        ps_acc[dt][:],
                        lhsT=w2[:, ft * D + dt * 128:ft * D + (dt + 1) * 128],
                        rhs=ffs[:], start=(ft == 0), stop=(ft == FT - 1),
                        skip_group_check=True)
            for dt in range(DC):
                nc.vector.scalar_tensor_tensor(
                    outT[dt][:], ps_acc[dt][:], consts[:, C_B2 + dt:C_B2 + dt + 1],
                    x2T[dt][:], op0=ALU.add, op1=ALU.add)

        # ---------------- store (transpose to token-major) ----------------
        with ExitStack() as s:
            ptr2 = s.enter_context(tc.tile_pool(name="trps2", bufs=2, space="PSUM"))
            posb2 = s.enter_context(tc.tile_pool(name="osbp2", bufs=2))
            QSZ = [128, 128, 128, 8]
            for qt in range(4):
                qsz = QSZ[qt]
                osb = posb2.tile([128, D], F32, tag="osb2", name="osb")
                for dt in range(DC):
                    tp = ptr2.tile([128, 128], F32, tag="tp", name="tp")
                    nc.tensor.transpose(tp[0:qsz, :],
                                        outT[dt][:, qt * 128:qt * 128 + qsz],
                                        ident[:])
                    if dt % 2 == 0:
                        nc.scalar.copy(osb[0:qsz, dt * 128:(dt + 1) * 128],
                                       tp[0:qsz, :])
                    else:
                        nc.vector.tensor_copy(osb[0:qsz, dt * 128:(dt + 1) * 128],
                                              tp[0:qsz, :])
                nc.sync.dma_start(out_d[qt * 128:qt * 128 + qsz, :], osb[0:qsz, :])

    nc.finalize()
    return nc


_NC = None


def _get_nc():
    global _NC
    if _NC is None:
        _NC = build_program()
    return _NC


def _stripes(mat, nstripe):
    """[nstripe*128, C] -> [128, nstripe*C] with stripe i at cols [i*C,(i+1)*C)."""
    r, c = mat.shape
    assert r == nstripe * 128
    return np.ascontiguousarray(
        mat.reshape(nstripe, 128, c).transpose(1, 0, 2).reshape(128, nstripe * c))


def _q_rows(hf):
    """Reordered query rows (within x, 0-based): [f4|f5|f6|f7] x 98."""
    return np.concatenate([np.arange(98) + f * NPATCH + hf * 98
                           for f in range(4)])


def _host_prepare(inputs):
    f32 = np.float32
    bf16 = ml_dtypes.bfloat16
    fp8 = ml_dtypes.float8_e4m3
    x = np.asarray(inputs["x"], f32)
    memory = np.asarray(inputs["memory"], f32)
    w_qkv = np.asarray(inputs["w_qkv"], f32)
    w_out = np.asarray(inputs["w_out"], f32)
    b_out = np.asarray(inputs["b_out"], f32)
    g_att = np.asarray(inputs["ln_att_g"], f32)
    b_att = np.asarray(inputs["ln_att_b"], f32)
    g2 = np.asarray(inputs["ln2_g"], f32)
    bb2 = np.asarray(inputs["ln2_b"], f32)
    w1 = np.asarray(inputs["w1"], f32)
    b1 = np.asarray(inputs["b1"], f32)
    w2 = np.asarray(inputs["w2"], f32)
    b2v = np.asarray(inputs["b2"], f32)

    w_qkv_eff = w_qkv * g_att[None, :]
    cb_qkv = w_qkv @ b_att
    cb_q4 = (4.0 * cb_qkv[:D]).astype(f32)
    cb_v = cb_qkv[2 * D:].astype(f32)
    b_out_eff = (b_out + w_out @ cb_v).astype(f32)
    w1_eff = w1 * g2[None, :]
    cb1_eff = (w1 @ bb2 + b1).astype(f32)

    def cols(v):
        return np.ascontiguousarray(v.reshape(-1, 128).T)

    shared = {
        "wq": _stripes(np.ascontiguousarray(4.0 * w_qkv_eff.T), DC).astype(fp8),
        "wout": _stripes(np.ascontiguousarray(4.0 * w_out.T), DC).astype(fp8),
        "w1": _stripes(np.ascontiguousarray(w1_eff.T), DC).astype(bf16),
        "w2": _stripes(np.ascontiguousarray(w2.T), FT).astype(bf16),
        "bvec": np.ascontiguousarray((64.0 * b_out_eff)[None, :]).astype(bf16),
    }
    cc = np.zeros((128, NCONST), f32)
    cc[:, C_CBQ:C_CBQ + DC] = cols(cb_q4)
    cc[:, C_B2:C_B2 + DC] = cols(b2v)
    cc[:, C_CB1:C_CB1 + FT] = cols(cb1_eff)
    cc[:, C_SCO] = 1.0 / 64.0
    cc[:, C_BB] = -4.0
    p = np.arange(128)
    for zc, (b0, _, _) in enumerate([ZEROPASS[7], ZEROPASS[9], ZEROPASS[10]]):
        cc[:, C_ZP + zc] = (p < b0).astype(f32)
    shared["consts"] = cc

    in_maps = []
    for c in range(NCORES):
        b, hf = divmod(c, 2)
        x_aug = np.concatenate([memory[b, :T], x[b]], axis=0)      # [L, D]
        xT = x_aug.T    # [768, 1568] -> chunk-major [128, sum(DC*LCH)]
        xall_np = np.concatenate(
            [xT[dc * 128:(dc + 1) * 128, ci * 512:ci * 512 + LCH[ci]]
             for ci in range(4) for dc in range(DC)], axis=1)
        q = x_aug[T + _q_rows(hf)]                                 # [NQ, D]
        in_maps.append({
            "xall": np.ascontiguousarray(xall_np).astype(fp8),
            "xq": _stripes(np.ascontiguousarray(q.T), DC).astype(bf16),
            **shared,
        })
    return in_maps


def _assemble(results):
    out = np.zeros((B, T, D), np.float32)
    for c in range(NCORES):
        b, hf = divmod(c, 2)
        out[b, _q_rows(hf), :] = results[c]["out"]
    return out


def kernel(**inputs):
    nc = _get_nc()
    in_maps = _host_prepare(inputs)
    res = run_bass_kernel_spmd(nc, in_maps, list(range(NCORES)))
    return _assemble(res.results)


def _ensure_ntff_hook():
    """Provide antenv.axon_hooks (absent in this image) so trace=True can
    drive NTFF capture through libaxon_pjrt.so, mirroring trn_boot.py."""
    import contextlib
    import ctypes
    import types

    try:
        from antenv.axon_hooks import get_axon_ntff_profile_hook  # noqa: F401
        return
    except ImportError:
        pass
    import antenv

    so_path = "/opt/axon/libaxon_pjrt.so"
    lib = ctypes.CDLL(so_path)
    if not hasattr(lib, "axon_start_nrt_profile"):
        raise RuntimeError("libaxon_pjrt.so lacks NTFF profile symbols")
    lib.axon_start_nrt_profile.argtypes = [ctypes.POINTER(ctypes.c_int64),
                                           ctypes.c_size_t]
    lib.axon_start_nrt_profile.restype = ctypes.c_int64
    lib.axon_stop_nrt_profile.argtypes = [ctypes.c_char_p]
    lib.axon_stop_nrt_profile.restype = ctypes.c_int64

    @contextlib.contextmanager
    def _hook(output_dir, device_ids):
        import jax
        jax.devices()
        if device_ids:
            ids = (ctypes.c_int64 * len(device_ids))(*device_ids)
            rc = lib.axon_start_nrt_profile(ids, len(device_ids))
        else:
            rc = lib.axon_start_nrt_profile(None, 0)
        if rc != 0:
            raise RuntimeError(f"axon_start_nrt_profile rc={rc}")
        try:
            yield
        finally:
            n = lib.axon_stop_nrt_profile(str(output_dir).encode())
            print(f"ntff profile: {n} file(s) written to {output_dir}",
                  file=sys.stderr)

    box = {"h": _hook}
    mod = types.ModuleType("antenv.axon_hooks")
    mod.set_axon_ntff_profile_hook = lambda h: box.__setitem__("h", h)
    mod.get_axon_ntff_profile_hook = lambda: box["h"]
    sys.modules["antenv.axon_hooks"] = mod
    antenv.axon_hooks = mod


def kernel_traced(**inputs):
    """Like kernel() but with NTFF profiling; returns (out, exec_time_ns)."""
    import tempfile

    from concourse import bass_utils as _bu
    _ensure_ntff_hook()
    _bu.upload_artifacts = lambda tmpdir: f"local:{tmpdir}"  # no bucket creds here
    nc = _get_nc()
    in_maps = _host_prepare(inputs)
    tmpdir = tempfile.mkdtemp(prefix="ntff_")
    res = run_bass_kernel_spmd(nc, in_maps, list(range(NCORES)), trace=True,
                               tmpdir=tmpdir)
    return _assemble(res.results), res.exec_time_ns
